# revision 1
# baseline (speedup 1.0000x reference)
"""Additive noise channel kernel for 8 Trainium2 NeuronCores.

Computes out[b, s, 0:2] = complex_FIR(x, a)[b, s] + (L @ (scale * noise))[b, s]
with B=64, S=8192, T=129 taps, L lower-triangular [S, S].

Strategy
--------
The dominant cost is reading L (256 MB fp32, half of it zeros).  We shard the
OUTPUT dim S across the 8 cores so each core reads only its columns of L^T,
and we exploit the triangular structure with a staircase assignment that is
perfectly SPMD-uniform: core k takes the eight 128-column strips
beta = 8j + k (j = 0..7).  Strip slot j is padded to a uniform extent of
8*(j+1) k-tiles of 128 rows (provably the minimal uniform cover of the
triangle), so every core runs the identical instruction stream on 288
k-tiles of packed L^T (vs 512 for a naive row shard, 4x that for the
batch-parallel hint).  L^T is carried in fp8e3m4 (pre-scaled by C_LT, the
inverse folded into the fp16 noise stationary at zero cost), everything
else fp16, accumulation fp32 in PSUM: ~1.3e-3 scaled absmax output error --
below what a plain all-bf16 kernel produces on this problem.

On-device everything is TensorE matmuls accumulating in PSUM:
  * noise coloring: lhsT = [scale*noise_r^T | scale*noise_i^T]  (K=128, M=128)
                    rhs  = L^T tile (fp8)                        (K=128, N=128)
    -> psum rows 0:64 = real part, rows 64:128 = imag part; one stream of L
    feeds both real and imag outputs.
  * complex FIR: expressed as x_ext^T @ A where A is the banded Toeplitz
    matrix of the taps, folded into the same PSUM accumulation
    (yr = xr*Ar - xi*Ai, yi = xr*Ai + xi*Ar); the second stationary
    [-xi | xr] is derived on the otherwise-idle VectorE.

Schedule: window-pair-major -- pair p covers k-tiles [16p, 16p+16) of every
still-active slot, so the noise-stationary demand spreads evenly instead of
front-loading; completed slots evacuate + stream out mid-kernel (completing
slots go first within pairs 1-3 so their chains overlap the pair's stream;
pair 0's go last because their FIR needs the late-arriving constants), and
slots 6/7's FIR runs a pair early, so the tail after the last chunk is one
short matmul chain.  The fs/a2/npk constant loads are pinned behind specific
chunks with sync=False dependency edges: without them the Tile scheduler
hoists these dep-free loads ahead of the chunk stream (6.6 us PE stall);
anchored too early they displace pair-0 chunk bytes (1 us PE stall) -- the
swept optimum anchors fs/a2 behind chunks 3/4 and the three noise-window
prefetches behind chunks 7/12/16.

All DRAM inputs are packed host-side in SBUF-image layout (partition-major,
2-4 KB contiguous runs per partition, chunk sequence in exact consumption
order) so the HBM read stream is sequential and every DMA descriptor is
>=1 KB.  Outputs are written planar (real / imag) and interleaved on the
host via one merged planar tensor (row = plane*B + batch, matching the psum
partition layout, so each store is a single full-128-partition DMA).
Cost-model timeline: 28.5 us/core, 0.5 us above the analytic lower bound
for any schedule of this decomposition (max over chunks of arrival time +
remaining PE work, plus the copy/store/sem/barrier constants).
"""

import os
import sys
import time

for _p in ("/opt/trn_rl_repo", "/root/.axon_site/_ro/trn_rl_repo"):
    if _p not in sys.path:
        sys.path.append(_p)

# the bass kernel executes through jax/PJRT on the axon-tunneled NeuronCores
os.environ.setdefault("JAX_PLATFORMS", "axon,cpu")

import numpy as np

import concourse.bass as bass
import concourse.mybir as mybir
import concourse.tile as tile
from concourse.tile import add_dep_helper
from concourse import bacc
from concourse.bass_utils import run_bass_kernel_spmd

B = 64          # batch
S = 8192        # block size
T = 129         # taps
H = (T - 1) // 2  # 64
P = 128         # partitions / k-tile
N_CORES = 8
N_SLOTS = 8     # strips per core
W = 128         # strip width (output columns per slot)
SLOT_KT = [8 * (j + 1) for j in range(N_SLOTS)]   # padded k-tiles per slot
TOT_KT = sum(SLOT_KT)  # 288

# Window-pair-major schedule: pair p covers k-tiles [16p, 16p+16).  All slots
# still alive advance through that window together, so the npk (noise) demand
# spreads evenly across the kernel instead of front-loading, and slots 2p /
# 2p+1 finish in pair p (their outputs stream out mid-kernel).
# CONSUME entries: (slot j, first k-tile kt0, n k-tiles ck, flat offset);
# chunks are laid out back-to-back in DRAM in this (consumption) order.
CONSUME = []
_flat = 0
for _p in range(4):
    # pairs 1-3: completing slots FIRST -- their chunks arrive earliest in
    # the pair, so their FIR + psum evacuation + store all overlap the rest
    # of the pair's chunk stream instead of gating the kernel tail.  Pair 0
    # keeps them LAST: slots 0/1's FIR needs the fs/a2/fsi constants, which
    # only land a few us in.
    if _p == 0:
        _order = list(range(2, N_SLOTS)) + [0, 1]
    elif _p == 3:
        # slot 7 last, with its final chunk split so the chain after the
        # very last byte is only 4 matmuls + one 64 KB store
        _order = [6, 7]
    else:
        _order = [2 * _p, 2 * _p + 1] + list(range(2 * _p + 2, N_SLOTS))
    for _j in _order:
        _ck = 8 if _j == 2 * _p else 16
        CONSUME.append((_j, 16 * _p, _ck, _flat))
        _flat += _ck
assert _flat == TOT_KT
_j9, _kt9, _ck9, _fl9 = CONSUME[-1]
CONSUME[-1:] = [(_j9, _kt9, 12, _fl9), (_j9, _kt9 + 12, 4, _fl9 + 12)]

# Precision mode.  "mixed8": L^T in fp8e3m4 (pre-scaled by C_LT, folded back
# via the fp16 noise stationary), everything else fp16, fp32 PSUM accumulate
# -> ~1.3e-3 scaled absmax error, below a plain all-bf16 kernel's error.
# "float16": all operands fp16 (~3e-4).  "float32": exact (~2e-7), 4x slower.
NOISE_DT = "mixed8"

C_LT = 64.0  # fp8 pre-scale: lt stores C_LT*L^T, npk stores scale*noise/C_LT

_DT_NP = {"float32": np.float32, "float16": np.float16}


def _mode_dtypes(dt_name):
    """returns (lt mybir dt, operand mybir dt name) for a mode."""
    if dt_name == "mixed8":
        return "float8e3", "float16"
    return dt_name, dt_name

LAST_RUN_SECONDS = None
_CACHE = {}


def _build_program(dt_name: str):
    lt_dt_name, op_dt_name = _mode_dtypes(dt_name)
    lt_dt = getattr(mybir.dt, lt_dt_name)
    dt = getattr(mybir.dt, op_dt_name)
    f32 = mybir.dt.float32

    nc = bacc.Bacc("TRN2", target_bir_lowering=False, debug=False,
                   num_devices=N_CORES)

    # all inputs are SBUF images: [128 partitions, free...]; lt is a flat
    # sequence of per-chunk SBUF images in consumption order
    lt = nc.dram_tensor("lt", [TOT_KT * P * P], lt_dt, kind="ExternalInput")
    npk = nc.dram_tensor("npk", [P, S // P, P], dt, kind="ExternalInput")
    fs = nc.dram_tensor("fs", [P, N_SLOTS * 2, P], dt, kind="ExternalInput")
    a2 = nc.dram_tensor("a2", [P, 2, 2, P], dt, kind="ExternalInput")
    # single planar output: row = plane*B + batch (plane 0 = real, 1 = imag)
    # -- matches the psum/staging partition layout, so every store is one
    # full-128-partition DMA instead of two 64-partition ones
    out2 = nc.dram_tensor("out2", [2 * B, N_SLOTS * W], f32,
                          kind="ExternalOutput")

    with tile.TileContext(nc) as tc:
        with (
            tc.tile_pool(name="const", bufs=1) as const,
            tc.tile_pool(name="ltp", bufs=9) as ltp,
            tc.tile_pool(name="psum", bufs=1, space=bass.MemorySpace.PSUM) as psum,
            tc.tile_pool(name="stage", bufs=1) as stage,
        ):
            # npk streams in window-sized pieces as the pairs consume it; the
            # first pieces go on the scalar ring so chunk 0 leads the sync
            # ring and the first matmul starts as early as possible.
            npk_sb = const.tile([P, S // P, P], dt)
            nc.scalar.dma_start(npk_sb[:, 0:8, :], npk.ap()[:, 0:8, :])
            nc.scalar.dma_start(npk_sb[:, 8:16, :], npk.ap()[:, 8:16, :])
            fs_sb = const.tile([P, N_SLOTS * 2, P], dt)
            a2_sb = const.tile([P, 2, 2, P], dt)
            fsi_sb = const.tile([P, N_SLOTS * 2, P], dt)

            ps = [psum.tile([P, W], f32, name=f"acc{j}", tag=f"acc{j}")
                  for j in range(N_SLOTS)]
            st = stage.tile([P, 6, W], f32)
            stB = stage.tile([P, 2, W], f32)
            n_dma = 0
            npk_prefetch = {7: (16, 32), 12: (32, 48), 16: (48, 64)}

            def chunk_dma(n_chunk, ck, flat):
                nonlocal n_dma
                ltc = ltp.tile([P, 16, P], lt_dt, tag="lt", name=f"lt{n_chunk}")
                dma_eng = nc.sync if n_dma % 2 == 0 else nc.scalar
                n_dma += 1
                chunk_inst = dma_eng.dma_start(
                    ltc[:, :ck, :],
                    lt.ap()[flat * P * P:(flat + ck) * P * P].rearrange(
                        "(p n m) -> p n m", p=P, n=ck))
                # fs/a2 aren't needed until the first slots complete at the
                # end of pair 0 -- keep them (and the npk prefetches) behind
                # early chunks with explicit edges so the scheduler can't
                # hoist these dep-free const loads ahead of the chunk stream.
                if n_chunk == 3:
                    fs_inst = nc.sync.dma_start(fs_sb[:], fs.ap())
                    add_dep_helper(fs_inst.ins, chunk_inst.ins, sync=False,
                                   reason="defer fs behind first chunk")
                if n_chunk == 4:
                    a2_inst = dma_eng.dma_start(a2_sb[:], a2.ap())
                    add_dep_helper(a2_inst.ins, chunk_inst.ins, sync=False,
                                   reason="defer a2 behind chunk")
                    # slots complete in ascending order -> derive ascending
                    for g in range(N_SLOTS * 2):
                        nc.vector.tensor_scalar_mul(fsi_sb[:, g, 0:B],
                                                    fs_sb[:, g, B:2 * B], -1.0)
                        nc.vector.tensor_copy(fsi_sb[:, g, B:2 * B],
                                              fs_sb[:, g, 0:B])
                # prefetch the next pair's noise window mid-pair
                if n_chunk in npk_prefetch:
                    lo, hi = npk_prefetch[n_chunk]
                    pf_inst = dma_eng.dma_start(npk_sb[:, lo:hi, :],
                                                npk.ap()[:, lo:hi, :])
                    add_dep_helper(pf_inst.ins, chunk_inst.ins, sync=False,
                                   reason="defer npk prefetch behind chunk")
                return ltc

            def fir_mms(j, stop):
                # FIR: stream A_r against [xr|xi], A_i against [-xi|xr]
                for sdx in (0, 1):
                    for c in (0, 1):
                        g = j * 2 + c
                        src = fs_sb if sdx == 0 else fsi_sb
                        nc.tensor.matmul(ps[j][:], src[:, g, :],
                                         a2_sb[:, sdx, c, :],
                                         start=False,
                                         stop=(stop and sdx == 1 and c == 1))

            def finish_slot(j):
                # slot j's accumulation is complete: evacuate and stream out
                dst = st[:, j, :] if j < 6 else stB[:, j - 6, :]
                nc.vector.tensor_copy(dst, ps[j][:])


            for n_chunk, (j, kt0, ck, flat) in enumerate(CONSUME):
                ltc = chunk_dma(n_chunk, ck, flat)
                # slots 6/7: their FIR only needs fs/a2, so it runs during
                # pair 2, shortening the serial chain after the last chunk
                fir_early = j >= 6 and kt0 == 32
                last_wins_stop = not (j >= 6)
                for i in range(ck):
                    is_last = kt0 + ck == SLOT_KT[j] and i == ck - 1
                    nc.tensor.matmul(ps[j][:], npk_sb[:, kt0 + i, :],
                                     ltc[:, i, :],
                                     start=(kt0 + i == 0),
                                     stop=(is_last and not last_wins_stop))
                if fir_early:
                    fir_mms(j, stop=False)
                if kt0 + ck == SLOT_KT[j]:
                    if last_wins_stop:
                        fir_mms(j, stop=True)
                    finish_slot(j)
            # all stores emitted after the load stream so they never steal
            # DMA-engine time from the chunk loads; the first two fire as
            # soon as their copies land (in the loads' natural gaps)
            nc.sync.dma_start(out2.ap()[:, :4 * W],
                              st[:, 0:4].rearrange("p j w -> p (j w)"))
            nc.scalar.dma_start(out2.ap()[:, 4 * W:6 * W],
                                st[:, 4:6].rearrange("p j w -> p (j w)"))
            nc.scalar.dma_start(out2.ap()[:, 7 * W:], stB[:, 1, :])
            nc.sync.dma_start(out2.ap()[:, 6 * W:7 * W], stB[:, 0, :])

    nc.compile()
    return nc


def _sbuf_image(arr_ktpm):
    """[nkt*128, m] k-tile-major -> SBUF image [128, nkt*m]."""
    nktp, m = arr_ktpm.shape
    nkt = nktp // P
    return np.ascontiguousarray(
        arr_ktpm.reshape(nkt, P, m).transpose(1, 0, 2).reshape(P, nkt * m))


def _prep_inputs(x_real, x_imag, a_real, a_imag, L, noise_r, noise_i, N0,
                 dt_name: str):
    mixed8 = dt_name == "mixed8"
    if mixed8:
        import ml_dtypes
        np_dt = np.float16
        lt_np_dt = ml_dtypes.float8_e3m4
        lt_scale, npk_scale = np.float32(C_LT), np.float32(1.0 / C_LT)
    else:
        np_dt = _DT_NP[dt_name]
        lt_np_dt = np_dt
        lt_scale, npk_scale = np.float32(1.0), np.float32(1.0)

    scale = np.float32(np.sqrt(0.5 * np.power(10.0, np.float64(N0[0]) / 10.0)))

    # packed scaled noise [S, 128]: cols 0:64 real, 64:128 imag
    npk = np.empty((S, 2 * B), np.float32)
    npk[:, :B] = (npk_scale * scale * noise_r).T
    npk[:, B:] = (npk_scale * scale * noise_i).T
    npk = _sbuf_image(npk.astype(np_dt)).reshape(P, S // P, P)

    # x transposed and zero-padded by H on both sides: row r <-> x col r - H
    xpad = np.zeros((S + 2 * H, 2 * B), np.float32)
    xpad[H:H + S, :B] = x_real.T
    xpad[H:H + S, B:] = x_imag.T
    xpad = xpad.astype(np_dt)

    # banded Toeplitz of the taps: A[r, j] = a[j + 2H - r] (valid range only)
    a2 = np.zeros((2, 2 * P, P), np.float32)
    rr = np.arange(2 * P)[:, None]
    jj = np.arange(W)[None, :]
    tap_idx = jj + 2 * H - rr
    valid = (tap_idx >= 0) & (tap_idx < T)
    a2[0][valid] = np.asarray(a_real, np.float32)[tap_idx[valid]]
    a2[1][valid] = np.asarray(a_imag, np.float32)[tap_idx[valid]]
    a2 = _sbuf_image(a2.reshape(2 * 2 * P, P).astype(np_dt)).reshape(P, 2, 2, P)

    L = np.asarray(L, np.float32)

    in_maps = []
    for k in range(N_CORES):
        ltpack = np.zeros((TOT_KT * P * P,), lt_np_dt)
        for j, kt0, ck, flat in CONSUME:
            beta = 8 * j + k
            rows_real = P * (beta + 1)     # non-zero extent in t of strip beta
            r0 = P * kt0                   # this chunk covers t rows r0:r1
            nreal = min(max(rows_real - r0, 0), ck * P)
            if nreal <= 0:
                continue
            block = np.zeros((ck * P, W), lt_np_dt)
            block[:nreal] = np.asarray(
                lt_scale * L[P * beta:P * (beta + 1), r0:r0 + nreal],
                lt_np_dt).T
            img = block.reshape(ck, P, W).transpose(1, 0, 2)
            ltpack[flat * P * P:(flat + ck) * P * P] = img.ravel()

        fsk = np.empty((N_SLOTS * 2, P, 2 * B), np_dt)
        for j in range(N_SLOTS):
            s0 = P * (8 * j + k)           # global first output col of slot
            fsk[j * 2] = xpad[s0:s0 + P]           # [xr | xi] k-tile 0
            fsk[j * 2 + 1] = xpad[s0 + P:s0 + 2 * P]  # k-tile 1
        fsk = _sbuf_image(fsk.reshape(N_SLOTS * 2 * P, 2 * B)).reshape(
            P, N_SLOTS * 2, P)
        in_maps.append({"lt": ltpack, "npk": npk, "fs": fsk, "a2": a2})
    return in_maps


def kernel(x_real, x_imag, a_real, a_imag, L, noise_r, noise_i, N0):
    global LAST_RUN_SECONDS
    inputs = dict(x_real=np.asarray(x_real, np.float32),
                  x_imag=np.asarray(x_imag, np.float32),
                  a_real=np.asarray(a_real, np.float32),
                  a_imag=np.asarray(a_imag, np.float32),
                  L=np.asarray(L, np.float32),
                  noise_r=np.asarray(noise_r, np.float32),
                  noise_i=np.asarray(noise_i, np.float32),
                  N0=np.asarray(N0, np.float32))

    if NOISE_DT not in _CACHE:
        _CACHE[NOISE_DT] = _build_program(NOISE_DT)
    nc = _CACHE[NOISE_DT]

    in_maps = _prep_inputs(**inputs, dt_name=NOISE_DT)

    t0 = time.time()
    res = run_bass_kernel_spmd(nc, in_maps, core_ids=list(range(N_CORES)))
    LAST_RUN_SECONDS = time.time() - t0

    planar = np.empty((2, B, N_SLOTS, N_CORES, W), np.float32)
    for k in range(N_CORES):
        o = res.results[k]["out2"].reshape(2, B, N_SLOTS, W)
        planar[0, :, :, k] = o[0]
        planar[1, :, :, k] = o[1]
    full = np.empty((B, S, 2), np.float32)
    full[:, :, 0] = planar[0].reshape(B, S)
    full[:, :, 1] = planar[1].reshape(B, S)
    return full



# revision 11
# speedup vs baseline: 1.6499x; 1.6499x over previous
"""Additive noise channel kernel for 8 Trainium2 NeuronCores.

Computes out[b, s, 0:2] = complex_FIR(x, a)[b, s] + (L @ (scale * noise))[b, s]
with B=64, S=8192, T=129 taps, L lower-triangular [S, S].

Strategy
--------
The dominant cost is reading L (256 MB fp32, half of it zeros).  We shard the
OUTPUT dim S across the 8 cores so each core reads only its columns of L^T,
and we exploit the triangular structure with a staircase assignment that is
perfectly SPMD-uniform: core k takes the eight 128-column strips
beta = 8j + k (j = 0..7).  Strip slot j is padded to a uniform extent of
8*(j+1) k-tiles of 128 rows (provably the minimal uniform cover of the
triangle), so every core runs the identical instruction stream on 288
k-tiles of packed L^T (vs 512 for a naive row shard, 4x that for the
batch-parallel hint).  L^T is carried in fp8e3m4 (pre-scaled by C_LT, the
inverse folded into the fp16 noise stationary at zero cost), everything
else fp16, accumulation fp32 in PSUM: ~1.3e-3 scaled absmax output error --
below what a plain all-bf16 kernel produces on this problem.

On-device everything is TensorE matmuls accumulating in PSUM:
  * noise coloring: lhsT = [scale*noise_r^T | scale*noise_i^T]  (K=128, M=128)
                    rhs  = L^T tile (fp8)                        (K=128, N=128)
    -> psum rows 0:64 = real part, rows 64:128 = imag part; one stream of L
    feeds both real and imag outputs.
  * complex FIR: expressed as x_ext^T @ A where A is the banded Toeplitz
    matrix of the taps, folded into the same PSUM accumulation
    (yr = xr*Ar - xi*Ai, yi = xr*Ai + xi*Ar); the second stationary
    [-xi | xr] is derived on the otherwise-idle VectorE.

Schedule: window-pair-major -- pair p covers k-tiles [16p, 16p+16) of every
still-active slot, so the noise-stationary demand spreads evenly instead of
front-loading; completed slots evacuate + stream out mid-kernel (completing
slots go first within pairs 1-3 so their chains overlap the pair's stream;
pair 0's go last because their FIR needs the late-arriving constants), and
slots 6/7's FIR runs a pair early, so the tail after the last chunk is one
short matmul chain.  The fs/a2/npk constant loads are pinned behind specific
chunks with sync=False dependency edges: without them the Tile scheduler
hoists these dep-free loads ahead of the chunk stream (6.6 us PE stall);
anchored too early they displace pair-0 chunk bytes (1 us PE stall) -- the
swept optimum anchors fs/a2 behind chunks 3/4 and the three noise-window
prefetches behind chunks 7/12/16.

All DRAM inputs are packed host-side in SBUF-image layout (partition-major,
2-4 KB contiguous runs per partition, chunk sequence in exact consumption
order) so the HBM read stream is sequential and every DMA descriptor is
>=1 KB.  Outputs are written planar (real / imag) and interleaved on the
host via one merged planar tensor (row = plane*B + batch, matching the psum
partition layout, so each store is a single full-128-partition DMA).
Cost-model timeline: 28.5 us/core, 0.5 us above the analytic lower bound
for any schedule of this decomposition (max over chunks of arrival time +
remaining PE work, plus the copy/store/sem/barrier constants).
"""

import os
import sys
import time

for _p in ("/opt/trn_rl_repo", "/root/.axon_site/_ro/trn_rl_repo"):
    if _p not in sys.path:
        sys.path.append(_p)

# the bass kernel executes through jax/PJRT on the axon-tunneled NeuronCores
os.environ.setdefault("JAX_PLATFORMS", "axon,cpu")

import numpy as np

import concourse.bass as bass
import concourse.mybir as mybir
import concourse.tile as tile
from concourse.tile import add_dep_helper
from concourse import bacc
from concourse.bass_utils import run_bass_kernel_spmd

B = 64          # batch
S = 8192        # block size
T = 129         # taps
H = (T - 1) // 2  # 64
P = 128         # partitions / k-tile
N_CORES = 8
N_SLOTS = 8     # strips per core
W = 128         # strip width (output columns per slot)
SLOT_KT = [8 * (j + 1) for j in range(N_SLOTS)]   # padded k-tiles per slot
TOT_KT = sum(SLOT_KT)  # 288

# Window-pair-major schedule: pair p covers k-tiles [16p, 16p+16).  All slots
# still alive advance through that window together, so the npk (noise) demand
# spreads evenly across the kernel instead of front-loading, and slots 2p /
# 2p+1 finish in pair p (their outputs stream out mid-kernel).
# CONSUME entries: (slot j, first k-tile kt0, n k-tiles ck, flat offset);
# chunks are laid out back-to-back in DRAM in this (consumption) order.
CONSUME = []
_flat = 0
for _p in range(4):
    # pairs 1-3: completing slots FIRST -- their chunks arrive earliest in
    # the pair, so their FIR + psum evacuation + store all overlap the rest
    # of the pair's chunk stream instead of gating the kernel tail.  Pair 0
    # keeps them LAST: slots 0/1's FIR needs the fs/a2/fsi constants, which
    # only land a few us in.
    if _p == 0:
        _order = list(range(2, N_SLOTS)) + [0, 1]
    elif _p == 3:
        # slot 7 last, with its final chunk split so the chain after the
        # very last byte is only 4 matmuls + one 64 KB store
        _order = [6, 7]
    else:
        _order = [2 * _p, 2 * _p + 1] + list(range(2 * _p + 2, N_SLOTS))
    for _j in _order:
        _ck = 8 if _j == 2 * _p else 16
        CONSUME.append((_j, 16 * _p, _ck, _flat))
        _flat += _ck
assert _flat == TOT_KT
_j9, _kt9, _ck9, _fl9 = CONSUME[-1]
CONSUME[-1:] = [(_j9, _kt9, 12, _fl9), (_j9, _kt9 + 12, 4, _fl9 + 12)]

# Precision mode.  "dr8": L^T AND the noise both in fp8e4m3 so every noise
# matmul runs in DoubleRow perf mode (two k-tiles per instruction, 0.5
# cycles/row); FIR stays fp16; stores fp16.  lt is pre-scaled by C_L=64 (kept
# in e4m3's sweet spot), the noise NOT divided by it; instead the FIR taps
# are pre-scaled by C_L and the psum evacuation multiplies by 1/C_L, which
# costs nothing (tensor_scalar_mul replaces the tensor_copy).
# "mixed8": L^T in fp8e3m4 (pre-scaled by C_LT, folded back via the fp16
# noise stationary), everything else fp16, fp32 PSUM accumulate.
# "float16": all operands fp16 (~3e-4).  "float32": exact (~2e-7), 4x slower.
NOISE_DT = "dr8"

C_LT = 64.0  # fp8 pre-scale: lt stores C_LT*L^T, npk stores scale*noise/C_LT

_DT_NP = {"float32": np.float32, "float16": np.float16}


def _mode_dtypes(dt_name):
    """returns (lt mybir dt, operand mybir dt name) for a mode."""
    if dt_name == "dr8":
        return "float8e4", "float16"
    if dt_name == "mixed8":
        return "float8e3", "float16"
    return dt_name, dt_name

LAST_RUN_SECONDS = None
_CACHE = {}

# ---------------------------------------------------------------------------
# "dr15" mode: zero-padding 15-slot split-strip layout + DoubleRow fp8e4.
#
# Strip beta (0..63, output cols [128b, 128(b+1))) has beta+1 nonzero k-tiles;
# write beta+1 = 8m + r (r in 1..8).  Split it into a BOTTOM piece (k-tiles
# [0, 8m), pure noise partial) and a TOP piece (k-tiles [8m, 8m+r), includes
# the diagonal + the FIR).  Per core: 7 "m-slots" of sizes 8m (m=1..7), one
# per bottom piece of strips {8m + k}, and 8 "r-slots" of sizes r (r=1..8),
# the top pieces of strips {8k + r - 1}.  Total = exactly 260 k-tiles per
# core -- the 28-tile SPMD padding of the 8-slot staircase is gone.  The two
# partials of each strip land on different cores; the host adds them during
# the unshard (it is already gathering anyway).
#
# npk locals: [0, 8) = the per-core window (global k-tiles [8k, 8k+8), used
# by the r-slots, whose global positions are core-dependent), [8, 64) =
# globals [0, 56) (used by the m-slots, core-invariant).  The duplication
# costs 16 KB and buys a uniform instruction stream.
#
# Windows: w = 0..6 processes m-slot k-tiles [8w, 8(w+1)) of every m-slot
# still alive, so m-slot (w+1) completes in window w, plus r-slot (w+1)'s
# whole top piece; window 6 also runs r8 and ENDS with m7's last tiles so
# the tail after the final chunk is evac + one small store.
# ---------------------------------------------------------------------------
N_SLOT15 = 15
M_SLOTS = list(range(1, 8))   # sizes 8m
R_SLOTS = list(range(1, 9))   # sizes r
TOT15 = sum(8 * m for m in M_SLOTS) + sum(R_SLOTS)  # 260

# storage/completion order: m1 r1 m2 r2 ... m6 r6 r7 r8 m7
SLOT15_ORDER = []
for _v in range(1, 7):
    SLOT15_ORDER += [("m", _v), ("r", _v)]
SLOT15_ORDER += [("r", 7), ("r", 8), ("m", 7)]
SLOT15_COL = {s: i for i, s in enumerate(SLOT15_ORDER)}

CHUNK15 = 32  # k-tiles per lt DMA chunk


def _stream15():
    """Consumption stream: list of windows; each window is an ordered list
    of items:
      ("mm", slot, npk_local, sz, start, stop)  -- noise matmul unit
      ("fir", r, stop_on_fir)                   -- 4 FIR matmuls for r-slot r
      ("fin", slot)                             -- psum evacuation
      ("store", lo, hi)                         -- staging cols [lo, hi) out
    Noise units consume sz k-tiles of the flat lt stream in order."""
    windows = []
    started = set()

    def units(slot, npk_lo, n, last_stops):
        out = []
        lo = 0
        while lo < n:
            sz = 2 if n - lo >= 2 else 1
            st = slot not in started
            started.add(slot)
            stop = last_stops and lo + sz == n
            out.append(("mm", slot, npk_lo + lo, sz, st, stop))
            lo += sz
        return out

    for w in range(6):
        win = []
        win += units(("m", w + 1), 8 + 8 * w, 8, True)
        win.append(("fin", ("m", w + 1)))
        win += units(("r", w + 1), 0, w + 1, False)
        for mh in range(w + 2, 8):
            win += units(("m", mh), 8 + 8 * w, 8, False)
        win.append(("fir", w + 1, True))
        win.append(("fin", ("r", w + 1)))
        if w in (1, 3):
            win.append(("store", 2 * w - 2, 2 * w + 2))
        windows.append(win)
    # window 6: r7, r8 complete; m7 goes LAST so the tail is short
    win = []
    win += units(("r", 7), 0, 7, False)
    win += units(("r", 8), 0, 8, False)
    win.append(("fir", 7, True))
    win.append(("fin", ("r", 7)))
    win.append(("fir", 8, True))
    win.append(("fin", ("r", 8)))
    win.append(("store", 8, 12))
    win += units(("m", 7), 8 + 48, 8, True)
    win.append(("fin", ("m", 7)))
    win.append(("store", 12, 15))
    windows.append(win)
    return windows


def _chunks15():
    """Split the 260-tile lt stream into DMA chunks at unit boundaries.
    Returns (chunk_sizes, unit_chunk_pos): for each noise unit (in stream
    order) the (chunk_idx, offset) its lt tiles live at."""
    sizes, pos = [], []
    cur = 0
    for win in _stream15():
        for it in win:
            if it[0] != "mm":
                continue
            sz = it[3]
            if cur + sz > CHUNK15 or not sizes:
                sizes.append(0)
                cur = 0
            pos.append((len(sizes) - 1, cur))
            sizes[-1] += sz
            cur += sz
    assert sum(sizes) == TOT15
    return sizes, pos


def _build_program(dt_name: str):
    dr8 = dt_name == "dr8"
    lt_dt_name, op_dt_name = _mode_dtypes(dt_name)
    lt_dt = getattr(mybir.dt, lt_dt_name)
    dt = getattr(mybir.dt, op_dt_name)
    npk_dt = mybir.dt.float8e4 if dr8 else dt
    st_dt = mybir.dt.float16 if dr8 else mybir.dt.float32
    f32 = mybir.dt.float32

    nc = bacc.Bacc("TRN2", target_bir_lowering=False, debug=False,
                   num_devices=N_CORES)

    # all inputs are SBUF images: [128 partitions, free...]; lt is a flat
    # sequence of per-chunk SBUF images in consumption order
    lt = nc.dram_tensor("lt", [TOT_KT * P * P], lt_dt, kind="ExternalInput")
    npk = nc.dram_tensor("npk", [P, S // P, P], npk_dt, kind="ExternalInput")
    fs = nc.dram_tensor("fs", [P, N_SLOTS * 2, P], dt, kind="ExternalInput")
    a2 = nc.dram_tensor("a2", [P, 2, 2, P], dt, kind="ExternalInput")
    # single planar output: row = plane*B + batch (plane 0 = real, 1 = imag)
    # -- matches the psum/staging partition layout, so every store is one
    # full-128-partition DMA instead of two 64-partition ones
    out2 = nc.dram_tensor("out2", [2 * B, N_SLOTS * W], st_dt,
                          kind="ExternalOutput")

    with tile.TileContext(nc) as tc:
        with (
            tc.tile_pool(name="const", bufs=1) as const,
            tc.tile_pool(name="ltp", bufs=9) as ltp,
            tc.tile_pool(name="psum", bufs=1, space=bass.MemorySpace.PSUM) as psum,
            tc.tile_pool(name="stage", bufs=1) as stage,
        ):
            # npk streams in window-sized pieces as the pairs consume it; the
            # first pieces go on the scalar ring so chunk 0 leads the sync
            # ring and the first matmul starts as early as possible.
            npk_sb = const.tile([P, S // P, P], npk_dt)
            nc.scalar.dma_start(npk_sb[:, 0:8, :], npk.ap()[:, 0:8, :])
            nc.scalar.dma_start(npk_sb[:, 8:16, :], npk.ap()[:, 8:16, :])
            fs_sb = const.tile([P, N_SLOTS * 2, P], dt)
            a2_sb = const.tile([P, 2, 2, P], dt)
            fsi_sb = const.tile([P, N_SLOTS * 2, P], dt)

            ps = [psum.tile([P, W], f32, name=f"acc{j}", tag=f"acc{j}")
                  for j in range(N_SLOTS)]
            st = stage.tile([P, 6, W], st_dt)
            stB = stage.tile([P, 2, W], st_dt)
            n_dma = 0
            npk_prefetch = {7: (16, 32), 12: (32, 48), 16: (48, 64)}

            def chunk_dma(n_chunk, ck, flat):
                nonlocal n_dma
                ltc = ltp.tile([P, 16, P], lt_dt, tag="lt", name=f"lt{n_chunk}")
                dma_eng = nc.sync if n_dma % 2 == 0 else nc.scalar
                n_dma += 1
                chunk_inst = dma_eng.dma_start(
                    ltc[:, :ck, :],
                    lt.ap()[flat * P * P:(flat + ck) * P * P].rearrange(
                        "(p n m) -> p n m", p=P, n=ck))
                # fs/a2 aren't needed until the first slots complete at the
                # end of pair 0 -- keep them (and the npk prefetches) behind
                # early chunks with explicit edges so the scheduler can't
                # hoist these dep-free const loads ahead of the chunk stream.
                if n_chunk == 3:
                    fs_inst = nc.sync.dma_start(fs_sb[:], fs.ap())
                    add_dep_helper(fs_inst.ins, chunk_inst.ins, sync=False,
                                   reason="defer fs behind first chunk")
                if n_chunk == 4:
                    a2_inst = dma_eng.dma_start(a2_sb[:], a2.ap())
                    add_dep_helper(a2_inst.ins, chunk_inst.ins, sync=False,
                                   reason="defer a2 behind chunk")
                    # slots complete in ascending order -> derive ascending
                    for g in range(N_SLOTS * 2):
                        nc.vector.tensor_scalar_mul(fsi_sb[:, g, 0:B],
                                                    fs_sb[:, g, B:2 * B], -1.0)
                        nc.vector.tensor_copy(fsi_sb[:, g, B:2 * B],
                                              fs_sb[:, g, 0:B])
                # prefetch the next pair's noise window mid-pair
                if n_chunk in npk_prefetch:
                    lo, hi = npk_prefetch[n_chunk]
                    pf_inst = dma_eng.dma_start(npk_sb[:, lo:hi, :],
                                                npk.ap()[:, lo:hi, :])
                    add_dep_helper(pf_inst.ins, chunk_inst.ins, sync=False,
                                   reason="defer npk prefetch behind chunk")
                return ltc

            def fir_mms(j, stop):
                # FIR: stream A_r against [xr|xi], A_i against [-xi|xr]
                for sdx in (0, 1):
                    for c in (0, 1):
                        g = j * 2 + c
                        src = fs_sb if sdx == 0 else fsi_sb
                        nc.tensor.matmul(ps[j][:], src[:, g, :],
                                         a2_sb[:, sdx, c, :],
                                         start=False,
                                         stop=(stop and sdx == 1 and c == 1))

            def finish_slot(j):
                # slot j's accumulation is complete: evacuate and stream out
                # (dr8: the 1/C_LT that undoes the lt pre-scale rides along)
                dst = st[:, j, :] if j < 6 else stB[:, j - 6, :]
                if dr8:
                    nc.vector.tensor_scalar_mul(dst, ps[j][:], 1.0 / C_LT)
                else:
                    nc.vector.tensor_copy(dst, ps[j][:])


            for n_chunk, (j, kt0, ck, flat) in enumerate(CONSUME):
                ltc = chunk_dma(n_chunk, ck, flat)
                # slots 6/7: their FIR only needs fs/a2, so it runs during
                # pair 2, shortening the serial chain after the last chunk
                fir_early = j >= 6 and kt0 == 32
                last_wins_stop = not (j >= 6)
                if dr8:
                    # DoubleRow: one matmul per PAIR of k-tiles (both
                    # operands fp8e4) at 0.5 cycles/row
                    for i in range(0, ck, 2):
                        is_last = kt0 + ck == SLOT_KT[j] and i == ck - 2
                        nc.tensor.matmul(
                            ps[j][:], npk_sb[:, kt0 + i:kt0 + i + 2, :],
                            ltc[:, i:i + 2, :],
                            start=(kt0 + i == 0),
                            stop=(is_last and not last_wins_stop),
                            perf_mode=mybir.MatmulPerfMode.DoubleRow)
                else:
                    for i in range(ck):
                        is_last = kt0 + ck == SLOT_KT[j] and i == ck - 1
                        nc.tensor.matmul(ps[j][:], npk_sb[:, kt0 + i, :],
                                         ltc[:, i, :],
                                         start=(kt0 + i == 0),
                                         stop=(is_last and not last_wins_stop))
                if fir_early:
                    fir_mms(j, stop=False)
                if kt0 + ck == SLOT_KT[j]:
                    if last_wins_stop:
                        fir_mms(j, stop=True)
                    finish_slot(j)
            # all stores emitted after the load stream so they never steal
            # DMA-engine time from the chunk loads; the first two fire as
            # soon as their copies land (in the loads' natural gaps)
            nc.sync.dma_start(out2.ap()[:, :4 * W],
                              st[:, 0:4].rearrange("p j w -> p (j w)"))
            nc.scalar.dma_start(out2.ap()[:, 4 * W:6 * W],
                                st[:, 4:6].rearrange("p j w -> p (j w)"))
            nc.scalar.dma_start(out2.ap()[:, 7 * W:], stB[:, 1, :])
            nc.sync.dma_start(out2.ap()[:, 6 * W:7 * W], stB[:, 0, :])

    nc.compile()
    return nc


def _sbuf_image(arr_ktpm):
    """[nkt*128, m] k-tile-major -> SBUF image [128, nkt*m]."""
    nktp, m = arr_ktpm.shape
    nkt = nktp // P
    return np.ascontiguousarray(
        arr_ktpm.reshape(nkt, P, m).transpose(1, 0, 2).reshape(P, nkt * m))


def _prep_inputs(x_real, x_imag, a_real, a_imag, L, noise_r, noise_i, N0,
                 dt_name: str):
    mixed8 = dt_name == "mixed8"
    dr8 = dt_name == "dr8"
    a2_scale = np.float32(1.0)
    if dr8:
        # lt holds C_LT*L^T in e4m3; noise is NOT pre-divided (it would land
        # in e4m3's subnormal range) -- instead the taps absorb C_LT and the
        # psum evacuation multiplies everything by 1/C_LT.
        import ml_dtypes
        np_dt = np.float16
        npk_np_dt = ml_dtypes.float8_e4m3
        lt_np_dt = ml_dtypes.float8_e4m3
        lt_scale, npk_scale = np.float32(C_LT), np.float32(1.0)
        a2_scale = np.float32(C_LT)
    elif mixed8:
        import ml_dtypes
        np_dt = np.float16
        npk_np_dt = np_dt
        lt_np_dt = ml_dtypes.float8_e3m4
        lt_scale, npk_scale = np.float32(C_LT), np.float32(1.0 / C_LT)
    else:
        np_dt = _DT_NP[dt_name]
        npk_np_dt = np_dt
        lt_np_dt = np_dt
        lt_scale, npk_scale = np.float32(1.0), np.float32(1.0)

    scale = np.float32(np.sqrt(0.5 * np.power(10.0, np.float64(N0[0]) / 10.0)))

    # packed scaled noise [S, 128]: cols 0:64 real, 64:128 imag
    npk = np.empty((S, 2 * B), np.float32)
    npk[:, :B] = (npk_scale * scale * noise_r).T
    npk[:, B:] = (npk_scale * scale * noise_i).T
    npk = _sbuf_image(npk.astype(npk_np_dt)).reshape(P, S // P, P)

    # x transposed and zero-padded by H on both sides: row r <-> x col r - H
    xpad = np.zeros((S + 2 * H, 2 * B), np.float32)
    xpad[H:H + S, :B] = x_real.T
    xpad[H:H + S, B:] = x_imag.T
    xpad = xpad.astype(np_dt)

    # banded Toeplitz of the taps: A[r, j] = a[j + 2H - r] (valid range only)
    a2 = np.zeros((2, 2 * P, P), np.float32)
    rr = np.arange(2 * P)[:, None]
    jj = np.arange(W)[None, :]
    tap_idx = jj + 2 * H - rr
    valid = (tap_idx >= 0) & (tap_idx < T)
    a2[0][valid] = a2_scale * np.asarray(a_real, np.float32)[tap_idx[valid]]
    a2[1][valid] = a2_scale * np.asarray(a_imag, np.float32)[tap_idx[valid]]
    a2 = _sbuf_image(a2.reshape(2 * 2 * P, P).astype(np_dt)).reshape(P, 2, 2, P)

    L = np.asarray(L, np.float32)

    in_maps = []
    for k in range(N_CORES):
        ltpack = np.zeros((TOT_KT * P * P,), lt_np_dt)
        for j, kt0, ck, flat in CONSUME:
            beta = 8 * j + k
            rows_real = P * (beta + 1)     # non-zero extent in t of strip beta
            r0 = P * kt0                   # this chunk covers t rows r0:r1
            nreal = min(max(rows_real - r0, 0), ck * P)
            if nreal <= 0:
                continue
            block = np.zeros((ck * P, W), lt_np_dt)
            block[:nreal] = np.asarray(
                lt_scale * L[P * beta:P * (beta + 1), r0:r0 + nreal],
                lt_np_dt).T
            img = block.reshape(ck, P, W).transpose(1, 0, 2)
            ltpack[flat * P * P:(flat + ck) * P * P] = img.ravel()

        fsk = np.empty((N_SLOTS * 2, P, 2 * B), np_dt)
        for j in range(N_SLOTS):
            s0 = P * (8 * j + k)           # global first output col of slot
            fsk[j * 2] = xpad[s0:s0 + P]           # [xr | xi] k-tile 0
            fsk[j * 2 + 1] = xpad[s0 + P:s0 + 2 * P]  # k-tile 1
        fsk = _sbuf_image(fsk.reshape(N_SLOTS * 2 * P, 2 * B)).reshape(
            P, N_SLOTS * 2, P)
        in_maps.append({"lt": ltpack, "npk": npk, "fs": fsk, "a2": a2})
    return in_maps


def kernel(x_real, x_imag, a_real, a_imag, L, noise_r, noise_i, N0):
    global LAST_RUN_SECONDS
    inputs = dict(x_real=np.asarray(x_real, np.float32),
                  x_imag=np.asarray(x_imag, np.float32),
                  a_real=np.asarray(a_real, np.float32),
                  a_imag=np.asarray(a_imag, np.float32),
                  L=np.asarray(L, np.float32),
                  noise_r=np.asarray(noise_r, np.float32),
                  noise_i=np.asarray(noise_i, np.float32),
                  N0=np.asarray(N0, np.float32))

    if NOISE_DT not in _CACHE:
        _CACHE[NOISE_DT] = _build_program(NOISE_DT)
    nc = _CACHE[NOISE_DT]

    in_maps = _prep_inputs(**inputs, dt_name=NOISE_DT)

    t0 = time.time()
    res = run_bass_kernel_spmd(nc, in_maps, core_ids=list(range(N_CORES)))
    LAST_RUN_SECONDS = time.time() - t0

    planar = np.empty((2, B, N_SLOTS, N_CORES, W), np.float32)
    for k in range(N_CORES):
        o = res.results[k]["out2"].reshape(2, B, N_SLOTS, W)
        planar[0, :, :, k] = o[0]
        planar[1, :, :, k] = o[1]
    full = np.empty((B, S, 2), np.float32)
    full[:, :, 0] = planar[0].reshape(B, S)
    full[:, :, 1] = planar[1].reshape(B, S)
    return full



# revision 33
# speedup vs baseline: 1.7476x; 1.0592x over previous
"""Additive noise channel kernel for 8 Trainium2 NeuronCores.

Computes out[b, s, 0:2] = complex_FIR(x, a)[b, s] + (L @ (scale * noise))[b, s]
with B=64, S=8192, T=129 taps, L lower-triangular [S, S].

Strategy
--------
The dominant cost is reading L (256 MB fp32, half of it zeros).  We shard the
OUTPUT dim S across the 8 cores so each core reads only its columns of L^T,
and we exploit the triangular structure with a staircase assignment that is
perfectly SPMD-uniform: core k takes the eight 128-column strips
beta = 8j + k (j = 0..7).  Strip slot j is padded to a uniform extent of
8*(j+1) k-tiles of 128 rows (provably the minimal uniform cover of the
triangle), so every core runs the identical instruction stream on 288
k-tiles of packed L^T (vs 512 for a naive row shard, 4x that for the
batch-parallel hint).  L^T is carried in fp8e3m4 (pre-scaled by C_LT, the
inverse folded into the fp16 noise stationary at zero cost), everything
else fp16, accumulation fp32 in PSUM: ~1.3e-3 scaled absmax output error --
below what a plain all-bf16 kernel produces on this problem.

On-device everything is TensorE matmuls accumulating in PSUM:
  * noise coloring: lhsT = [scale*noise_r^T | scale*noise_i^T]  (K=128, M=128)
                    rhs  = L^T tile (fp8)                        (K=128, N=128)
    -> psum rows 0:64 = real part, rows 64:128 = imag part; one stream of L
    feeds both real and imag outputs.
  * complex FIR: expressed as x_ext^T @ A where A is the banded Toeplitz
    matrix of the taps, folded into the same PSUM accumulation
    (yr = xr*Ar - xi*Ai, yi = xr*Ai + xi*Ar); the second stationary
    [-xi | xr] is derived on the otherwise-idle VectorE.

Schedule: window-pair-major -- pair p covers k-tiles [16p, 16p+16) of every
still-active slot, so the noise-stationary demand spreads evenly instead of
front-loading; completed slots evacuate + stream out mid-kernel (completing
slots go first within pairs 1-3 so their chains overlap the pair's stream;
pair 0's go last because their FIR needs the late-arriving constants), and
slots 6/7's FIR runs a pair early, so the tail after the last chunk is one
short matmul chain.  The fs/a2/npk constant loads are pinned behind specific
chunks with sync=False dependency edges: without them the Tile scheduler
hoists these dep-free loads ahead of the chunk stream (6.6 us PE stall);
anchored too early they displace pair-0 chunk bytes (1 us PE stall) -- the
swept optimum anchors fs/a2 behind chunks 3/4 and the three noise-window
prefetches behind chunks 7/12/16.

All DRAM inputs are packed host-side in SBUF-image layout (partition-major,
2-4 KB contiguous runs per partition, chunk sequence in exact consumption
order) so the HBM read stream is sequential and every DMA descriptor is
>=1 KB.  Outputs are written planar (real / imag) and interleaved on the
host via one merged planar tensor (row = plane*B + batch, matching the psum
partition layout, so each store is a single full-128-partition DMA).
Cost-model timeline: 28.5 us/core, 0.5 us above the analytic lower bound
for any schedule of this decomposition (max over chunks of arrival time +
remaining PE work, plus the copy/store/sem/barrier constants).
"""

import os
import sys
import time

for _p in ("/opt/trn_rl_repo", "/root/.axon_site/_ro/trn_rl_repo"):
    if _p not in sys.path:
        sys.path.append(_p)

# the bass kernel executes through jax/PJRT on the axon-tunneled NeuronCores
os.environ.setdefault("JAX_PLATFORMS", "axon,cpu")

import numpy as np

import concourse.bass as bass
import concourse.mybir as mybir
import concourse.tile as tile
from concourse.tile import add_dep_helper
from concourse import bacc
from concourse.bass_utils import run_bass_kernel_spmd

B = 64          # batch
S = 8192        # block size
T = 129         # taps
H = (T - 1) // 2  # 64
P = 128         # partitions / k-tile
N_CORES = 8
N_SLOTS = 8     # strips per core
W = 128         # strip width (output columns per slot)
SLOT_KT = [8 * (j + 1) for j in range(N_SLOTS)]   # padded k-tiles per slot
TOT_KT = sum(SLOT_KT)  # 288

# Window-pair-major schedule: pair p covers k-tiles [16p, 16p+16).  All slots
# still alive advance through that window together, so the npk (noise) demand
# spreads evenly across the kernel instead of front-loading, and slots 2p /
# 2p+1 finish in pair p (their outputs stream out mid-kernel).
# CONSUME entries: (slot j, first k-tile kt0, n k-tiles ck, flat offset);
# chunks are laid out back-to-back in DRAM in this (consumption) order.
CONSUME = []
_flat = 0
for _p in range(4):
    # pairs 1-3: completing slots FIRST -- their chunks arrive earliest in
    # the pair, so their FIR + psum evacuation + store all overlap the rest
    # of the pair's chunk stream instead of gating the kernel tail.  Pair 0
    # keeps them LAST: slots 0/1's FIR needs the fs/a2/fsi constants, which
    # only land a few us in.
    if _p == 0:
        _order = list(range(2, N_SLOTS)) + [0, 1]
    elif _p == 3:
        # slot 7 last, with its final chunk split so the chain after the
        # very last byte is only 4 matmuls + one 64 KB store
        _order = [6, 7]
    else:
        _order = [2 * _p, 2 * _p + 1] + list(range(2 * _p + 2, N_SLOTS))
    for _j in _order:
        _ck = 8 if _j == 2 * _p else 16
        CONSUME.append((_j, 16 * _p, _ck, _flat))
        _flat += _ck
assert _flat == TOT_KT
_j9, _kt9, _ck9, _fl9 = CONSUME[-1]
CONSUME[-1:] = [(_j9, _kt9, 12, _fl9), (_j9, _kt9 + 12, 4, _fl9 + 12)]

# Precision mode.  "dr8": L^T AND the noise both in fp8e4m3 so every noise
# matmul runs in DoubleRow perf mode (two k-tiles per instruction, 0.5
# cycles/row); FIR stays fp16; stores fp16.  lt is pre-scaled by C_L=64 (kept
# in e4m3's sweet spot), the noise NOT divided by it; instead the FIR taps
# are pre-scaled by C_L and the psum evacuation multiplies by 1/C_L, which
# costs nothing (tensor_scalar_mul replaces the tensor_copy).
# "mixed8": L^T in fp8e3m4 (pre-scaled by C_LT, folded back via the fp16
# noise stationary), everything else fp16, fp32 PSUM accumulate.
# "float16": all operands fp16 (~3e-4).  "float32": exact (~2e-7), 4x slower.
NOISE_DT = "dr15"

C_LT = 64.0  # fp8 pre-scale: lt stores C_LT*L^T, npk stores scale*noise/C_LT

_DT_NP = {"float32": np.float32, "float16": np.float16}


def _mode_dtypes(dt_name):
    """returns (lt mybir dt, operand mybir dt name) for a mode."""
    if dt_name == "dr8":
        return "float8e4", "float16"
    if dt_name == "mixed8":
        return "float8e3", "float16"
    return dt_name, dt_name

LAST_RUN_SECONDS = None
_CACHE = {}

# ---------------------------------------------------------------------------
# "dr15" mode: zero-padding 15-slot split-strip layout + DoubleRow fp8e4.
#
# Strip beta (0..63, output cols [128b, 128(b+1))) has beta+1 nonzero k-tiles;
# write beta+1 = 8m + r (r in 1..8).  Split it into a BOTTOM piece (k-tiles
# [0, 8m), pure noise partial) and a TOP piece (k-tiles [8m, 8m+r), includes
# the diagonal + the FIR).  Per core: 7 "m-slots" of sizes 8m (m=1..7), one
# per bottom piece of strips {8m + k}, and 8 "r-slots" of sizes r (r=1..8),
# the top pieces of strips {8k + r - 1}.  Total = exactly 260 k-tiles per
# core -- the 28-tile SPMD padding of the 8-slot staircase is gone.  The two
# partials of each strip land on different cores; the host adds them during
# the unshard (it is already gathering anyway).
#
# npk locals: [0, 8) = the per-core window (global k-tiles [8k, 8k+8), used
# by the r-slots, whose global positions are core-dependent), [8, 64) =
# globals [0, 56) (used by the m-slots, core-invariant).  The duplication
# costs 16 KB and buys a uniform instruction stream.
#
# Slots run SEQUENTIALLY (segment-major) in ASCENDING m order with r-slots
# interleaved, so npk demand grows at the pace its pieces stream in, and at
# most ~3 psum accumulations are live at once (PSUM allocates at bank
# granularity: 8 x 2KB; pool cycles 6 bufs).  r1/r2's FIR is deferred until
# after m4 (the fsa/fsi constants only land a few us in); the stream ends
# with m7 whose tail is evacuate + one small store.
# ---------------------------------------------------------------------------
N_SLOT15 = 15
TOT15 = sum(8 * m for m in range(1, 8)) + sum(range(1, 9))  # 260

# completion order -> staging/out2 column
_COMPLETION15 = [("m", 1), ("m", 2), ("m", 3), ("r", 3), ("m", 4), ("r", 1),
                 ("r", 2), ("r", 4), ("m", 5), ("r", 5), ("m", 6), ("r", 6),
                 ("r", 7), ("r", 8), ("m", 7)]
SLOT15_COL = {s: i for i, s in enumerate(_COMPLETION15)}

CHUNK15 = 32  # k-tiles per lt DMA chunk


def _stream15():
    """Consumption stream: ordered items
      ("mm", slot, npk_local, sz, start, stop)  -- noise matmul unit
      ("fir", r, stop_on_fir)                   -- 4 FIR matmuls for r-slot r
      ("fin", slot)                             -- psum evacuation
      ("store", lo, hi)                         -- staging cols [lo, hi) out
    Noise units consume sz k-tiles of the flat lt stream in order.
    npk locals: m-slot m covers globals [0, 8m) = locals [8, 8+8m);
    r-slot r covers locals [0, r) (the per-core window)."""
    items = []

    def units(kind, v):
        n = 8 * v if kind == "m" else v
        npk0 = 8 if kind == "m" else 0
        lo = 0
        while lo < n:
            sz = 2 if n - lo >= 2 else 1
            stop = kind == "m" and lo + sz == n
            items.append(("mm", (kind, v), npk0 + lo, sz, lo == 0, stop))
            lo += sz

    def fir_fin(r):
        items.append(("fir", r, True))
        items.append(("fin", ("r", r)))

    def mseg(m):
        units("m", m)
        items.append(("fin", ("m", m)))

    mseg(1)
    units("r", 1)
    mseg(2)
    units("r", 2)
    mseg(3)
    units("r", 3)
    fir_fin(3)
    mseg(4)
    fir_fin(1)
    items.append(("store", 0, 4))
    fir_fin(2)
    units("r", 4)
    fir_fin(4)
    mseg(5)
    items.append(("store", 4, 8))
    units("r", 5)
    fir_fin(5)
    mseg(6)
    units("r", 6)
    fir_fin(6)
    units("r", 7)
    fir_fin(7)
    items.append(("store", 8, 12))
    units("r", 8)
    fir_fin(8)
    items.append(("store", 12, 14))
    mseg(7)
    items.append(("store", 14, 15))
    return items


def _chunks15():
    """Split the 260-tile lt stream into DMA chunks at unit boundaries.
    Returns (chunk_sizes, unit_chunk_pos): for each noise unit (in stream
    order) the (chunk_idx, offset) its lt tiles live at."""
    sizes, pos = [], []
    cur = 0
    consumed = 0
    for it in _stream15():
        if it[0] != "mm":
            continue
        sz = it[3]
        # small chunks at the very end keep the post-last-chunk chain short
        cap = CHUNK15 if TOT15 - consumed > 8 else 4
        if cur + sz > cap or not sizes:
            sizes.append(0)
            cur = 0
        pos.append((len(sizes) - 1, cur))
        sizes[-1] += sz
        cur += sz
        consumed += sz
    assert sum(sizes) == TOT15
    return sizes, pos


def _build_program15():
    """15-slot split-strip DoubleRow kernel (mode "dr15")."""
    fp8 = mybir.dt.float8e4
    fp16 = mybir.dt.float16
    f32 = mybir.dt.float32
    DR = mybir.MatmulPerfMode.DoubleRow

    nc = bacc.Bacc("TRN2", target_bir_lowering=False, debug=False,
                   num_devices=N_CORES)

    chunk_sizes, unit_pos = _chunks15()
    n_chunks = len(chunk_sizes)

    lt = nc.dram_tensor("lt", [TOT15 * P * P], fp8, kind="ExternalInput")
    npk = nc.dram_tensor("npk", [P, S // P, P], fp8, kind="ExternalInput")
    # fsa: 9 x-window images (cols 0..8) + 4 tap images (cols 9..12)
    fsa = nc.dram_tensor("fsa", [P, 13, P], fp16, kind="ExternalInput")
    out2 = nc.dram_tensor("out2", [2 * B, N_SLOT15 * P], fp16,
                          kind="ExternalOutput")

    with tile.TileContext(nc) as tc:
        with (
            tc.tile_pool(name="const", bufs=1) as const,
            tc.tile_pool(name="ltp", bufs=4) as ltp,
            tc.tile_pool(name="psum", bufs=6, space=bass.MemorySpace.PSUM) as psum,
            tc.tile_pool(name="stage", bufs=1) as stage,
        ):
            npk_sb = const.tile([P, S // P, P], fp8)
            fsa_sb = const.tile([P, 13, P], fp16)
            fsi_sb = const.tile([P, 9, P], fp16)
            nc.scalar.dma_start(npk_sb[:, 0:16, :], npk.ap()[:, 0:16, :])

            # psum tiles allocated lazily at first use; same tag -> the pool
            # cycles its 6 bufs in segment order (each reuse is of a slot
            # evacuated several segments earlier, so there is never a stall)
            ps = {}
            st = stage.tile([P, N_SLOT15, P], fp16)

            # chunk DMAs are emitted lazily as the stream consumes them so
            # the Tile scheduler sees them in consumption order
            lt_bufs = {}
            n_dma = 0

            def chunk_dma(ci):
                nonlocal n_dma
                ck = chunk_sizes[ci]
                flat = sum(chunk_sizes[:ci])
                ltc = ltp.tile([P, CHUNK15, P], fp8, tag="lt", name=f"lt{ci}")
                dma_eng = nc.sync if n_dma % 2 == 0 else nc.scalar
                n_dma += 1
                inst = dma_eng.dma_start(
                    ltc[:, :ck, :],
                    lt.ap()[flat * P * P:(flat + ck) * P * P].rearrange(
                        "(p n m) -> p n m", p=P, n=ck))
                # pin const loads behind early chunks so the scheduler can't
                # hoist them ahead of the byte stream
                if ci == 0:
                    fsa_inst = nc.sync.dma_start(fsa_sb[:], fsa.ap())
                    add_dep_helper(fsa_inst.ins, inst.ins, sync=False,
                                   reason="defer fsa behind chunk 0")
                    # derive [-xi | xr] from [xr | xi] in two strided ops
                    nc.vector.tensor_scalar_mul(fsi_sb[:, :, 0:B],
                                                fsa_sb[:, 0:9, B:2 * B], -1.0)
                    nc.vector.tensor_copy(fsi_sb[:, :, B:2 * B],
                                          fsa_sb[:, 0:9, 0:B])
                npk_pieces = {0: (16, 24), 1: (24, 40), 2: (40, 56),
                              3: (56, 64)}
                if ci in npk_pieces:
                    lo, hi = npk_pieces[ci]
                    pp = dma_eng.dma_start(npk_sb[:, lo:hi, :],
                                           npk.ap()[:, lo:hi, :])
                    add_dep_helper(pp.ins, inst.ins, sync=False,
                                   reason="defer npk piece behind chunk")
                return ltc

            n_store = 0
            unit_i = 0
            for it in _stream15():
                if it[0] == "mm":
                    _, slot, npk_lo, sz, start, stop = it
                    ci, off = unit_pos[unit_i]
                    unit_i += 1
                    if ci not in lt_bufs:
                        lt_bufs[ci] = chunk_dma(ci)
                    ltc = lt_bufs[ci]
                    if slot not in ps:
                        ps[slot] = psum.tile([P, P], f32, tag="ps",
                                             name=f"ps{slot[0]}{slot[1]}")
                    if sz == 2:
                        nc.tensor.matmul(
                            ps[slot][:],
                            npk_sb[:, npk_lo:npk_lo + 2, :],
                            ltc[:, off:off + 2, :],
                            start=start, stop=stop, perf_mode=DR)
                    else:
                        nc.tensor.matmul(
                            ps[slot][:], npk_sb[:, npk_lo, :],
                            ltc[:, off, :], start=start, stop=stop)
                elif it[0] == "fir":
                    _, r, stop_fir = it
                    b = r - 1
                    for sdx in (0, 1):
                        for c in (0, 1):
                            src = fsa_sb[:, b + c, :] if sdx == 0 \
                                else fsi_sb[:, b + c, :]
                            nc.tensor.matmul(
                                ps[("r", r)][:], src,
                                fsa_sb[:, 9 + 2 * sdx + c, :],
                                start=False,
                                stop=(stop_fir and sdx == 1 and c == 1))
                elif it[0] == "fin":
                    _, slot = it
                    nc.vector.tensor_scalar_mul(
                        st[:, SLOT15_COL[slot], :], ps[slot][:],
                        1.0 / C_LT)
                elif it[0] == "store":
                    _, lo, hi = it
                    eng = nc.sync if n_store % 2 == 0 else nc.scalar
                    n_store += 1
                    eng.dma_start(
                        out2.ap()[:, lo * P:hi * P],
                        st[:, lo:hi, :].rearrange("p j w -> p (j w)"))
            assert unit_i == len(unit_pos)

    nc.compile()
    return nc


def _build_program(dt_name: str):
    if dt_name == "dr15":
        return _build_program15()
    dr8 = dt_name == "dr8"
    lt_dt_name, op_dt_name = _mode_dtypes(dt_name)
    lt_dt = getattr(mybir.dt, lt_dt_name)
    dt = getattr(mybir.dt, op_dt_name)
    npk_dt = mybir.dt.float8e4 if dr8 else dt
    st_dt = mybir.dt.float16 if dr8 else mybir.dt.float32
    f32 = mybir.dt.float32

    nc = bacc.Bacc("TRN2", target_bir_lowering=False, debug=False,
                   num_devices=N_CORES)

    # all inputs are SBUF images: [128 partitions, free...]; lt is a flat
    # sequence of per-chunk SBUF images in consumption order
    lt = nc.dram_tensor("lt", [TOT_KT * P * P], lt_dt, kind="ExternalInput")
    npk = nc.dram_tensor("npk", [P, S // P, P], npk_dt, kind="ExternalInput")
    fs = nc.dram_tensor("fs", [P, N_SLOTS * 2, P], dt, kind="ExternalInput")
    a2 = nc.dram_tensor("a2", [P, 2, 2, P], dt, kind="ExternalInput")
    # single planar output: row = plane*B + batch (plane 0 = real, 1 = imag)
    # -- matches the psum/staging partition layout, so every store is one
    # full-128-partition DMA instead of two 64-partition ones
    out2 = nc.dram_tensor("out2", [2 * B, N_SLOTS * W], st_dt,
                          kind="ExternalOutput")

    with tile.TileContext(nc) as tc:
        with (
            tc.tile_pool(name="const", bufs=1) as const,
            tc.tile_pool(name="ltp", bufs=9) as ltp,
            tc.tile_pool(name="psum", bufs=1, space=bass.MemorySpace.PSUM) as psum,
            tc.tile_pool(name="stage", bufs=1) as stage,
        ):
            # npk streams in window-sized pieces as the pairs consume it; the
            # first pieces go on the scalar ring so chunk 0 leads the sync
            # ring and the first matmul starts as early as possible.
            npk_sb = const.tile([P, S // P, P], npk_dt)
            nc.scalar.dma_start(npk_sb[:, 0:8, :], npk.ap()[:, 0:8, :])
            nc.scalar.dma_start(npk_sb[:, 8:16, :], npk.ap()[:, 8:16, :])
            fs_sb = const.tile([P, N_SLOTS * 2, P], dt)
            a2_sb = const.tile([P, 2, 2, P], dt)
            fsi_sb = const.tile([P, N_SLOTS * 2, P], dt)

            ps = [psum.tile([P, W], f32, name=f"acc{j}", tag=f"acc{j}")
                  for j in range(N_SLOTS)]
            st = stage.tile([P, 6, W], st_dt)
            stB = stage.tile([P, 2, W], st_dt)
            n_dma = 0
            npk_prefetch = {7: (16, 32), 12: (32, 48), 16: (48, 64)}

            def chunk_dma(n_chunk, ck, flat):
                nonlocal n_dma
                ltc = ltp.tile([P, 16, P], lt_dt, tag="lt", name=f"lt{n_chunk}")
                dma_eng = nc.sync if n_dma % 2 == 0 else nc.scalar
                n_dma += 1
                chunk_inst = dma_eng.dma_start(
                    ltc[:, :ck, :],
                    lt.ap()[flat * P * P:(flat + ck) * P * P].rearrange(
                        "(p n m) -> p n m", p=P, n=ck))
                # fs/a2 aren't needed until the first slots complete at the
                # end of pair 0 -- keep them (and the npk prefetches) behind
                # early chunks with explicit edges so the scheduler can't
                # hoist these dep-free const loads ahead of the chunk stream.
                if n_chunk == 3:
                    fs_inst = nc.sync.dma_start(fs_sb[:], fs.ap())
                    add_dep_helper(fs_inst.ins, chunk_inst.ins, sync=False,
                                   reason="defer fs behind first chunk")
                if n_chunk == 4:
                    a2_inst = dma_eng.dma_start(a2_sb[:], a2.ap())
                    add_dep_helper(a2_inst.ins, chunk_inst.ins, sync=False,
                                   reason="defer a2 behind chunk")
                    # slots complete in ascending order -> derive ascending
                    for g in range(N_SLOTS * 2):
                        nc.vector.tensor_scalar_mul(fsi_sb[:, g, 0:B],
                                                    fs_sb[:, g, B:2 * B], -1.0)
                        nc.vector.tensor_copy(fsi_sb[:, g, B:2 * B],
                                              fs_sb[:, g, 0:B])
                # prefetch the next pair's noise window mid-pair
                if n_chunk in npk_prefetch:
                    lo, hi = npk_prefetch[n_chunk]
                    pf_inst = dma_eng.dma_start(npk_sb[:, lo:hi, :],
                                                npk.ap()[:, lo:hi, :])
                    add_dep_helper(pf_inst.ins, chunk_inst.ins, sync=False,
                                   reason="defer npk prefetch behind chunk")
                return ltc

            def fir_mms(j, stop):
                # FIR: stream A_r against [xr|xi], A_i against [-xi|xr]
                for sdx in (0, 1):
                    for c in (0, 1):
                        g = j * 2 + c
                        src = fs_sb if sdx == 0 else fsi_sb
                        nc.tensor.matmul(ps[j][:], src[:, g, :],
                                         a2_sb[:, sdx, c, :],
                                         start=False,
                                         stop=(stop and sdx == 1 and c == 1))

            def finish_slot(j):
                # slot j's accumulation is complete: evacuate and stream out
                # (dr8: the 1/C_LT that undoes the lt pre-scale rides along)
                dst = st[:, j, :] if j < 6 else stB[:, j - 6, :]
                if dr8:
                    nc.vector.tensor_scalar_mul(dst, ps[j][:], 1.0 / C_LT)
                else:
                    nc.vector.tensor_copy(dst, ps[j][:])


            for n_chunk, (j, kt0, ck, flat) in enumerate(CONSUME):
                ltc = chunk_dma(n_chunk, ck, flat)
                # slots 6/7: their FIR only needs fs/a2, so it runs during
                # pair 2, shortening the serial chain after the last chunk
                fir_early = j >= 6 and kt0 == 32
                last_wins_stop = not (j >= 6)
                if dr8:
                    # DoubleRow: one matmul per PAIR of k-tiles (both
                    # operands fp8e4) at 0.5 cycles/row
                    for i in range(0, ck, 2):
                        is_last = kt0 + ck == SLOT_KT[j] and i == ck - 2
                        nc.tensor.matmul(
                            ps[j][:], npk_sb[:, kt0 + i:kt0 + i + 2, :],
                            ltc[:, i:i + 2, :],
                            start=(kt0 + i == 0),
                            stop=(is_last and not last_wins_stop),
                            perf_mode=mybir.MatmulPerfMode.DoubleRow)
                else:
                    for i in range(ck):
                        is_last = kt0 + ck == SLOT_KT[j] and i == ck - 1
                        nc.tensor.matmul(ps[j][:], npk_sb[:, kt0 + i, :],
                                         ltc[:, i, :],
                                         start=(kt0 + i == 0),
                                         stop=(is_last and not last_wins_stop))
                if fir_early:
                    fir_mms(j, stop=False)
                if kt0 + ck == SLOT_KT[j]:
                    if last_wins_stop:
                        fir_mms(j, stop=True)
                    finish_slot(j)
            # all stores emitted after the load stream so they never steal
            # DMA-engine time from the chunk loads; the first two fire as
            # soon as their copies land (in the loads' natural gaps)
            nc.sync.dma_start(out2.ap()[:, :4 * W],
                              st[:, 0:4].rearrange("p j w -> p (j w)"))
            nc.scalar.dma_start(out2.ap()[:, 4 * W:6 * W],
                                st[:, 4:6].rearrange("p j w -> p (j w)"))
            nc.scalar.dma_start(out2.ap()[:, 7 * W:], stB[:, 1, :])
            nc.sync.dma_start(out2.ap()[:, 6 * W:7 * W], stB[:, 0, :])

    nc.compile()
    return nc


def _sbuf_image(arr_ktpm):
    """[nkt*128, m] k-tile-major -> SBUF image [128, nkt*m]."""
    nktp, m = arr_ktpm.shape
    nkt = nktp // P
    return np.ascontiguousarray(
        arr_ktpm.reshape(nkt, P, m).transpose(1, 0, 2).reshape(P, nkt * m))


def _prep_inputs(x_real, x_imag, a_real, a_imag, L, noise_r, noise_i, N0,
                 dt_name: str):
    mixed8 = dt_name == "mixed8"
    dr8 = dt_name == "dr8"
    a2_scale = np.float32(1.0)
    if dr8:
        # lt holds C_LT*L^T in e4m3; noise is NOT pre-divided (it would land
        # in e4m3's subnormal range) -- instead the taps absorb C_LT and the
        # psum evacuation multiplies everything by 1/C_LT.
        import ml_dtypes
        np_dt = np.float16
        npk_np_dt = ml_dtypes.float8_e4m3
        lt_np_dt = ml_dtypes.float8_e4m3
        lt_scale, npk_scale = np.float32(C_LT), np.float32(1.0)
        a2_scale = np.float32(C_LT)
    elif mixed8:
        import ml_dtypes
        np_dt = np.float16
        npk_np_dt = np_dt
        lt_np_dt = ml_dtypes.float8_e3m4
        lt_scale, npk_scale = np.float32(C_LT), np.float32(1.0 / C_LT)
    else:
        np_dt = _DT_NP[dt_name]
        npk_np_dt = np_dt
        lt_np_dt = np_dt
        lt_scale, npk_scale = np.float32(1.0), np.float32(1.0)

    scale = np.float32(np.sqrt(0.5 * np.power(10.0, np.float64(N0[0]) / 10.0)))

    # packed scaled noise [S, 128]: cols 0:64 real, 64:128 imag
    npk = np.empty((S, 2 * B), np.float32)
    npk[:, :B] = (npk_scale * scale * noise_r).T
    npk[:, B:] = (npk_scale * scale * noise_i).T
    npk = _sbuf_image(npk.astype(npk_np_dt)).reshape(P, S // P, P)

    # x transposed and zero-padded by H on both sides: row r <-> x col r - H
    xpad = np.zeros((S + 2 * H, 2 * B), np.float32)
    xpad[H:H + S, :B] = x_real.T
    xpad[H:H + S, B:] = x_imag.T
    xpad = xpad.astype(np_dt)

    # banded Toeplitz of the taps: A[r, j] = a[j + 2H - r] (valid range only)
    a2 = np.zeros((2, 2 * P, P), np.float32)
    rr = np.arange(2 * P)[:, None]
    jj = np.arange(W)[None, :]
    tap_idx = jj + 2 * H - rr
    valid = (tap_idx >= 0) & (tap_idx < T)
    a2[0][valid] = a2_scale * np.asarray(a_real, np.float32)[tap_idx[valid]]
    a2[1][valid] = a2_scale * np.asarray(a_imag, np.float32)[tap_idx[valid]]
    a2 = _sbuf_image(a2.reshape(2 * 2 * P, P).astype(np_dt)).reshape(P, 2, 2, P)

    L = np.asarray(L, np.float32)

    in_maps = []
    for k in range(N_CORES):
        ltpack = np.zeros((TOT_KT * P * P,), lt_np_dt)
        for j, kt0, ck, flat in CONSUME:
            beta = 8 * j + k
            rows_real = P * (beta + 1)     # non-zero extent in t of strip beta
            r0 = P * kt0                   # this chunk covers t rows r0:r1
            nreal = min(max(rows_real - r0, 0), ck * P)
            if nreal <= 0:
                continue
            block = np.zeros((ck * P, W), lt_np_dt)
            block[:nreal] = np.asarray(
                lt_scale * L[P * beta:P * (beta + 1), r0:r0 + nreal],
                lt_np_dt).T
            img = block.reshape(ck, P, W).transpose(1, 0, 2)
            ltpack[flat * P * P:(flat + ck) * P * P] = img.ravel()

        fsk = np.empty((N_SLOTS * 2, P, 2 * B), np_dt)
        for j in range(N_SLOTS):
            s0 = P * (8 * j + k)           # global first output col of slot
            fsk[j * 2] = xpad[s0:s0 + P]           # [xr | xi] k-tile 0
            fsk[j * 2 + 1] = xpad[s0 + P:s0 + 2 * P]  # k-tile 1
        fsk = _sbuf_image(fsk.reshape(N_SLOTS * 2 * P, 2 * B)).reshape(
            P, N_SLOTS * 2, P)
        in_maps.append({"lt": ltpack, "npk": npk, "fs": fsk, "a2": a2})
    return in_maps


def _prep_inputs15(x_real, x_imag, a_real, a_imag, L, noise_r, noise_i, N0):
    import ml_dtypes
    e4 = ml_dtypes.float8_e4m3
    fp16 = np.float16

    scale = np.float32(np.sqrt(0.5 * np.power(10.0, np.float64(N0[0]) / 10.0)))
    Lf = np.asarray(L, np.float32)

    # global scaled noise [S, 2B] in e4m3 (NOT divided by C_LT -- that would
    # land in e4m3's subnormal range; the taps absorb C_LT instead and the
    # psum evacuation multiplies by 1/C_LT)
    npk_g = np.empty((S, 2 * B), np.float32)
    npk_g[:, :B] = (scale * noise_r).T
    npk_g[:, B:] = (scale * noise_i).T
    npk_g = npk_g.astype(e4)

    # x transposed, offset by H=64: xpad[r] = x[r - 64]
    xpad = np.zeros((8320, 2 * B), np.float32)
    xpad[H:H + S, :B] = x_real.T
    xpad[H:H + S, B:] = x_imag.T
    xpad = xpad.astype(fp16)

    # banded Toeplitz taps, pre-scaled by C_LT: 4 images [128, 128]
    a2 = np.zeros((2, 2 * P, P), np.float32)
    rr = np.arange(2 * P)[:, None]
    jj = np.arange(P)[None, :]
    tap_idx = jj + 2 * H - rr
    valid = (tap_idx >= 0) & (tap_idx < T)
    a2[0][valid] = C_LT * np.asarray(a_real, np.float32)[tap_idx[valid]]
    a2[1][valid] = C_LT * np.asarray(a_imag, np.float32)[tap_idx[valid]]
    a2 = a2.reshape(4, P, P).astype(fp16)

    chunk_sizes, _ = _chunks15()
    in_maps = []
    for k in range(N_CORES):
        # npk locals: [0, 8) = globals [8k, 8k+8); [8, 64) = globals [0, 56)
        npk_loc = np.concatenate(
            [npk_g[1024 * k:1024 * (k + 1)], npk_g[:7168]])
        npk_img = _sbuf_image(npk_loc).reshape(P, S // P, P)

        # lt stream: per-unit blocks in consumption order
        tiles = []
        for it in _stream15():
            if it[0] != "mm":
                continue
            _, slot, npk_lo, sz, _, _ = it
            kind, v = slot
            if kind == "m":
                beta = 8 * v + k
                g0 = npk_lo - 8
            else:
                beta = 8 * k + v - 1
                g0 = 8 * k + npk_lo
            blk = (C_LT * Lf[128 * beta:128 * (beta + 1),
                             128 * g0:128 * (g0 + sz)]).T.astype(e4)
            tiles.append(blk.reshape(sz, P, P))
        tiles = np.concatenate(tiles)
        assert tiles.shape[0] == TOT15
        ltpack = np.empty((TOT15 * P * P,), e4)
        flat = 0
        for ck in chunk_sizes:
            ltpack[flat * P * P:(flat + ck) * P * P] = \
                tiles[flat:flat + ck].transpose(1, 0, 2).ravel()
            flat += ck

        # fsa: 9 x-window images + 4 tap images
        fsa = np.empty((13, P, P), fp16)
        for q in range(9):
            fsa[q] = xpad[1024 * k + 128 * q:1024 * k + 128 * (q + 1)]
        fsa[9:13] = a2
        fsa_img = _sbuf_image(fsa.reshape(13 * P, P)).reshape(P, 13, P)

        in_maps.append({"lt": ltpack, "npk": npk_img, "fsa": fsa_img})
    return in_maps


def _unshard15(results):
    """Add top+bottom partials per strip and reassemble [B, S, 2]."""
    outs = [np.asarray(results[k]["out2"], np.float32)
            for k in range(N_CORES)]
    acc = np.empty((2 * B, S), np.float32)
    for beta in range(64):
        m, r = beta // 8, beta % 8 + 1
        cols = slice(128 * beta, 128 * (beta + 1))
        tc = SLOT15_COL[("r", r)]
        acc[:, cols] = outs[m][:, 128 * tc:128 * (tc + 1)]
        if m >= 1:
            bc = SLOT15_COL[("m", m)]
            acc[:, cols] += outs[beta % 8][:, 128 * bc:128 * (bc + 1)]
    full = np.empty((B, S, 2), np.float32)
    full[:, :, 0] = acc[:B]
    full[:, :, 1] = acc[B:]
    return full


def kernel(x_real, x_imag, a_real, a_imag, L, noise_r, noise_i, N0):
    global LAST_RUN_SECONDS
    inputs = dict(x_real=np.asarray(x_real, np.float32),
                  x_imag=np.asarray(x_imag, np.float32),
                  a_real=np.asarray(a_real, np.float32),
                  a_imag=np.asarray(a_imag, np.float32),
                  L=np.asarray(L, np.float32),
                  noise_r=np.asarray(noise_r, np.float32),
                  noise_i=np.asarray(noise_i, np.float32),
                  N0=np.asarray(N0, np.float32))

    if NOISE_DT not in _CACHE:
        _CACHE[NOISE_DT] = _build_program(NOISE_DT)
    nc = _CACHE[NOISE_DT]

    if NOISE_DT == "dr15":
        in_maps = _prep_inputs15(**inputs)
        t0 = time.time()
        res = run_bass_kernel_spmd(nc, in_maps,
                                   core_ids=list(range(N_CORES)))
        LAST_RUN_SECONDS = time.time() - t0
        return _unshard15(res.results)

    in_maps = _prep_inputs(**inputs, dt_name=NOISE_DT)

    t0 = time.time()
    res = run_bass_kernel_spmd(nc, in_maps, core_ids=list(range(N_CORES)))
    LAST_RUN_SECONDS = time.time() - t0

    planar = np.empty((2, B, N_SLOTS, N_CORES, W), np.float32)
    for k in range(N_CORES):
        o = res.results[k]["out2"].reshape(2, B, N_SLOTS, W)
        planar[0, :, :, k] = o[0]
        planar[1, :, :, k] = o[1]
    full = np.empty((B, S, 2), np.float32)
    full[:, :, 0] = planar[0].reshape(B, S)
    full[:, :, 1] = planar[1].reshape(B, S)
    return full



# revision 52
# speedup vs baseline: 1.7681x; 1.0117x over previous
"""Additive noise channel kernel for 8 Trainium2 NeuronCores.

Computes out[b, s, 0:2] = complex_FIR(x, a)[b, s] + (L @ (scale * noise))[b, s]
with B=64, S=8192, T=129 taps, L lower-triangular [S, S].

Strategy
--------
The dominant cost is reading L (256 MB fp32, half of it zeros).  We shard the
OUTPUT dim S across the 8 cores so each core reads only its columns of L^T,
and we exploit the triangular structure with a staircase assignment that is
perfectly SPMD-uniform: core k takes the eight 128-column strips
beta = 8j + k (j = 0..7).  Strip slot j is padded to a uniform extent of
8*(j+1) k-tiles of 128 rows (provably the minimal uniform cover of the
triangle), so every core runs the identical instruction stream on 288
k-tiles of packed L^T (vs 512 for a naive row shard, 4x that for the
batch-parallel hint).  L^T is carried in fp8e3m4 (pre-scaled by C_LT, the
inverse folded into the fp16 noise stationary at zero cost), everything
else fp16, accumulation fp32 in PSUM: ~1.3e-3 scaled absmax output error --
below what a plain all-bf16 kernel produces on this problem.

On-device everything is TensorE matmuls accumulating in PSUM:
  * noise coloring: lhsT = [scale*noise_r^T | scale*noise_i^T]  (K=128, M=128)
                    rhs  = L^T tile (fp8)                        (K=128, N=128)
    -> psum rows 0:64 = real part, rows 64:128 = imag part; one stream of L
    feeds both real and imag outputs.
  * complex FIR: expressed as x_ext^T @ A where A is the banded Toeplitz
    matrix of the taps, folded into the same PSUM accumulation
    (yr = xr*Ar - xi*Ai, yi = xr*Ai + xi*Ar); the second stationary
    [-xi | xr] is derived on the otherwise-idle VectorE.

Schedule: window-pair-major -- pair p covers k-tiles [16p, 16p+16) of every
still-active slot, so the noise-stationary demand spreads evenly instead of
front-loading; completed slots evacuate + stream out mid-kernel (completing
slots go first within pairs 1-3 so their chains overlap the pair's stream;
pair 0's go last because their FIR needs the late-arriving constants), and
slots 6/7's FIR runs a pair early, so the tail after the last chunk is one
short matmul chain.  The fs/a2/npk constant loads are pinned behind specific
chunks with sync=False dependency edges: without them the Tile scheduler
hoists these dep-free loads ahead of the chunk stream (6.6 us PE stall);
anchored too early they displace pair-0 chunk bytes (1 us PE stall) -- the
swept optimum anchors fs/a2 behind chunks 3/4 and the three noise-window
prefetches behind chunks 7/12/16.

All DRAM inputs are packed host-side in SBUF-image layout (partition-major,
2-4 KB contiguous runs per partition, chunk sequence in exact consumption
order) so the HBM read stream is sequential and every DMA descriptor is
>=1 KB.  Outputs are written planar (real / imag) and interleaved on the
host via one merged planar tensor (row = plane*B + batch, matching the psum
partition layout, so each store is a single full-128-partition DMA).
Cost-model timeline: 28.5 us/core, 0.5 us above the analytic lower bound
for any schedule of this decomposition (max over chunks of arrival time +
remaining PE work, plus the copy/store/sem/barrier constants).
"""

import os
import sys
import time

for _p in ("/opt/trn_rl_repo", "/root/.axon_site/_ro/trn_rl_repo"):
    if _p not in sys.path:
        sys.path.append(_p)

# the bass kernel executes through jax/PJRT on the axon-tunneled NeuronCores
os.environ.setdefault("JAX_PLATFORMS", "axon,cpu")

import numpy as np

import concourse.bass as bass
import concourse.mybir as mybir
import concourse.tile as tile
from concourse.tile import add_dep_helper
from concourse import bacc
from concourse.bass_utils import run_bass_kernel_spmd

B = 64          # batch
S = 8192        # block size
T = 129         # taps
H = (T - 1) // 2  # 64
P = 128         # partitions / k-tile
N_CORES = 8
N_SLOTS = 8     # strips per core
W = 128         # strip width (output columns per slot)
SLOT_KT = [8 * (j + 1) for j in range(N_SLOTS)]   # padded k-tiles per slot
TOT_KT = sum(SLOT_KT)  # 288

# Window-pair-major schedule: pair p covers k-tiles [16p, 16p+16).  All slots
# still alive advance through that window together, so the npk (noise) demand
# spreads evenly across the kernel instead of front-loading, and slots 2p /
# 2p+1 finish in pair p (their outputs stream out mid-kernel).
# CONSUME entries: (slot j, first k-tile kt0, n k-tiles ck, flat offset);
# chunks are laid out back-to-back in DRAM in this (consumption) order.
CONSUME = []
_flat = 0
for _p in range(4):
    # pairs 1-3: completing slots FIRST -- their chunks arrive earliest in
    # the pair, so their FIR + psum evacuation + store all overlap the rest
    # of the pair's chunk stream instead of gating the kernel tail.  Pair 0
    # keeps them LAST: slots 0/1's FIR needs the fs/a2/fsi constants, which
    # only land a few us in.
    if _p == 0:
        _order = list(range(2, N_SLOTS)) + [0, 1]
    elif _p == 3:
        # slot 7 last, with its final chunk split so the chain after the
        # very last byte is only 4 matmuls + one 64 KB store
        _order = [6, 7]
    else:
        _order = [2 * _p, 2 * _p + 1] + list(range(2 * _p + 2, N_SLOTS))
    for _j in _order:
        _ck = 8 if _j == 2 * _p else 16
        CONSUME.append((_j, 16 * _p, _ck, _flat))
        _flat += _ck
assert _flat == TOT_KT
_j9, _kt9, _ck9, _fl9 = CONSUME[-1]
CONSUME[-1:] = [(_j9, _kt9, 12, _fl9), (_j9, _kt9 + 12, 4, _fl9 + 12)]

# Precision mode.  "dr8": L^T AND the noise both in fp8e4m3 so every noise
# matmul runs in DoubleRow perf mode (two k-tiles per instruction, 0.5
# cycles/row); FIR stays fp16; stores fp16.  lt is pre-scaled by C_L=64 (kept
# in e4m3's sweet spot), the noise NOT divided by it; instead the FIR taps
# are pre-scaled by C_L and the psum evacuation multiplies by 1/C_L, which
# costs nothing (tensor_scalar_mul replaces the tensor_copy).
# "mixed8": L^T in fp8e3m4 (pre-scaled by C_LT, folded back via the fp16
# noise stationary), everything else fp16, fp32 PSUM accumulate.
# "float16": all operands fp16 (~3e-4).  "float32": exact (~2e-7), 4x slower.
NOISE_DT = "dr15"

C_LT = 64.0  # fp8 pre-scale: lt stores C_LT*L^T, npk stores scale*noise/C_LT

_DT_NP = {"float32": np.float32, "float16": np.float16}


def _mode_dtypes(dt_name):
    """returns (lt mybir dt, operand mybir dt name) for a mode."""
    if dt_name == "dr8":
        return "float8e4", "float16"
    if dt_name == "mixed8":
        return "float8e3", "float16"
    return dt_name, dt_name

LAST_RUN_SECONDS = None
_CACHE = {}

# ---------------------------------------------------------------------------
# "dr15" mode: zero-padding 15-slot split-strip layout + DoubleRow fp8e4.
#
# Strip beta (0..63, output cols [128b, 128(b+1))) has beta+1 nonzero k-tiles;
# write beta+1 = 8m + r (r in 1..8).  Split it into a BOTTOM piece (k-tiles
# [0, 8m), pure noise partial) and a TOP piece (k-tiles [8m, 8m+r), includes
# the diagonal + the FIR).  Per core: 7 "m-slots" of sizes 8m (m=1..7), one
# per bottom piece of strips {8m + k}, and 8 "r-slots" of sizes r (r=1..8),
# the top pieces of strips {8k + r - 1}.  Total = exactly 260 k-tiles per
# core -- the 28-tile SPMD padding of the 8-slot staircase is gone.  The two
# partials of each strip land on different cores; the host adds them during
# the unshard (it is already gathering anyway).
#
# npk locals: [0, 8) = the per-core window (global k-tiles [8k, 8k+8), used
# by the r-slots, whose global positions are core-dependent), [8, 64) =
# globals [0, 56) (used by the m-slots, core-invariant).  The duplication
# costs 16 KB and buys a uniform instruction stream.
#
# Slots run SEQUENTIALLY (segment-major) in ASCENDING m order with r-slots
# interleaved, so npk demand grows at the pace its pieces stream in, and at
# most ~3 psum accumulations are live at once (PSUM allocates at bank
# granularity: 8 x 2KB; pool cycles 6 bufs).  r1/r2's FIR is deferred until
# after m4 (the fsa/fsi constants only land a few us in); the stream ends
# with m7 whose tail is evacuate + one small store.
# ---------------------------------------------------------------------------
N_SLOT15 = 15
TOT15 = sum(8 * m for m in range(1, 8)) + sum(range(1, 9))  # 260

# completion order -> staging/out2 column
_COMPLETION15 = [("m", 1), ("m", 2), ("m", 3), ("r", 3), ("m", 4), ("r", 1),
                 ("r", 2), ("r", 4), ("m", 5), ("r", 5), ("m", 6), ("r", 6),
                 ("r", 7), ("r", 8), ("m", 7)]
SLOT15_COL = {s: i for i, s in enumerate(_COMPLETION15)}

CHUNK15 = 40  # k-tiles per lt DMA chunk
# npk: leading load [0, NPK_P1_HI) fires before chunk 0; remaining pieces
# [lo, hi) are anchored behind lt chunk index ci (sync=False edges)
NPK_P1_HI = 64
NPK_PIECES15 = {}


def _stream15():
    """Consumption stream: ordered items
      ("mm", slot, npk_local, sz, start, stop)  -- noise matmul unit
      ("fir", r, stop_on_fir)                   -- 4 FIR matmuls for r-slot r
      ("fin", slot)                             -- psum evacuation
      ("store", lo, hi)                         -- staging cols [lo, hi) out
    Noise units consume sz k-tiles of the flat lt stream in order.
    npk locals: m-slot m covers globals [0, 8m) = locals [8, 8+8m);
    r-slot r covers locals [0, r) (the per-core window)."""
    items = []

    def units(kind, v):
        n = 8 * v if kind == "m" else v
        npk0 = 8 if kind == "m" else 0
        lo = 0
        while lo < n:
            sz = 2 if n - lo >= 2 else 1
            stop = kind == "m" and lo + sz == n
            items.append(("mm", (kind, v), npk0 + lo, sz, lo == 0, stop))
            lo += sz

    def fir_fin(r):
        items.append(("fir", r, True))
        items.append(("fin", ("r", r)))

    def mseg(m):
        units("m", m)
        items.append(("fin", ("m", m)))

    mseg(1)
    units("r", 1)
    mseg(2)
    units("r", 2)
    mseg(3)
    units("r", 3)
    fir_fin(3)
    mseg(4)
    fir_fin(1)
    items.append(("store", 0, 4))
    fir_fin(2)
    units("r", 4)
    fir_fin(4)
    mseg(5)
    items.append(("store", 4, 8))
    units("r", 5)
    fir_fin(5)
    mseg(6)
    units("r", 6)
    fir_fin(6)
    units("r", 7)
    fir_fin(7)
    items.append(("store", 8, 12))
    units("r", 8)
    fir_fin(8)
    items.append(("store", 12, 14))
    mseg(7)
    items.append(("store", 14, 15))
    return items


def _chunks15():
    """Split the 260-tile lt stream into DMA chunks at unit boundaries.
    Returns (chunk_sizes, unit_chunk_pos): for each noise unit (in stream
    order) the (chunk_idx, offset) its lt tiles live at."""
    sizes, pos = [], []
    cur = 0
    consumed = 0
    for it in _stream15():
        if it[0] != "mm":
            continue
        sz = it[3]
        # small chunks at the very end keep the post-last-chunk chain short
        cap = CHUNK15 if TOT15 - consumed > 8 else 4
        if cur + sz > cap or not sizes:
            sizes.append(0)
            cur = 0
        pos.append((len(sizes) - 1, cur))
        sizes[-1] += sz
        cur += sz
        consumed += sz
    assert sum(sizes) == TOT15
    return sizes, pos


def _build_program15():
    """15-slot split-strip DoubleRow kernel (mode "dr15")."""
    fp8 = mybir.dt.float8e4
    fp16 = mybir.dt.float16
    f32 = mybir.dt.float32
    DR = mybir.MatmulPerfMode.DoubleRow

    nc = bacc.Bacc("TRN2", target_bir_lowering=False, debug=False,
                   num_devices=N_CORES)

    chunk_sizes, unit_pos = _chunks15()
    n_chunks = len(chunk_sizes)

    lt = nc.dram_tensor("lt", [TOT15 * P * P], fp8, kind="ExternalInput")
    npk = nc.dram_tensor("npk", [P, S // P, P], fp8, kind="ExternalInput")
    # fsa: 9 x-window images (cols 0..8) + 4 tap images (cols 9..12)
    fsa = nc.dram_tensor("fsa", [P, 13, P], fp16, kind="ExternalInput")
    out2 = nc.dram_tensor("out2", [2 * B, N_SLOT15 * P], fp16,
                          kind="ExternalOutput")

    with tile.TileContext(nc) as tc:
        with (
            tc.tile_pool(name="const", bufs=1) as const,
            tc.tile_pool(name="ltp", bufs=4) as ltp,
            tc.tile_pool(name="psum", bufs=6, space=bass.MemorySpace.PSUM) as psum,
            tc.tile_pool(name="stage", bufs=1) as stage,
        ):
            npk_sb = const.tile([P, S // P, P], fp8)
            fsa_sb = const.tile([P, 13, P], fp16)
            fsi_sb = const.tile([P, 9, P], fp16)
            nc.scalar.dma_start(npk_sb[:, 0:NPK_P1_HI, :],
                                npk.ap()[:, 0:NPK_P1_HI, :])

            # psum tiles allocated lazily at first use; same tag -> the pool
            # cycles its 6 bufs in segment order (each reuse is of a slot
            # evacuated several segments earlier, so there is never a stall)
            ps = {}
            st = stage.tile([P, N_SLOT15, P], fp16)

            # chunk DMAs are emitted lazily as the stream consumes them so
            # the Tile scheduler sees them in consumption order
            lt_bufs = {}
            n_dma = 0

            def chunk_dma(ci):
                nonlocal n_dma
                ck = chunk_sizes[ci]
                flat = sum(chunk_sizes[:ci])
                ltc = ltp.tile([P, CHUNK15, P], fp8, tag="lt", name=f"lt{ci}")
                dma_eng = nc.sync if n_dma % 2 == 0 else nc.scalar
                n_dma += 1
                inst = dma_eng.dma_start(
                    ltc[:, :ck, :],
                    lt.ap()[flat * P * P:(flat + ck) * P * P].rearrange(
                        "(p n m) -> p n m", p=P, n=ck))
                # pin const loads behind early chunks so the scheduler can't
                # hoist them ahead of the byte stream
                if ci == 0:
                    fsa_inst = nc.sync.dma_start(fsa_sb[:], fsa.ap())
                    add_dep_helper(fsa_inst.ins, inst.ins, sync=False,
                                   reason="defer fsa behind chunk 0")
                    # derive [-xi | xr] from [xr | xi] in two strided ops
                    nc.vector.tensor_scalar_mul(fsi_sb[:, :, 0:B],
                                                fsa_sb[:, 0:9, B:2 * B], -1.0)
                    nc.vector.tensor_copy(fsi_sb[:, :, B:2 * B],
                                          fsa_sb[:, 0:9, 0:B])
                if ci in NPK_PIECES15:
                    lo, hi = NPK_PIECES15[ci]
                    pp = dma_eng.dma_start(npk_sb[:, lo:hi, :],
                                           npk.ap()[:, lo:hi, :])
                    add_dep_helper(pp.ins, inst.ins, sync=False,
                                   reason="defer npk piece behind chunk")
                return ltc

            n_store = 0
            unit_i = 0
            for it in _stream15():
                if it[0] == "mm":
                    _, slot, npk_lo, sz, start, stop = it
                    ci, off = unit_pos[unit_i]
                    unit_i += 1
                    if ci not in lt_bufs:
                        lt_bufs[ci] = chunk_dma(ci)
                    ltc = lt_bufs[ci]
                    if slot not in ps:
                        ps[slot] = psum.tile([P, P], f32, tag="ps",
                                             name=f"ps{slot[0]}{slot[1]}")
                    if sz == 2:
                        nc.tensor.matmul(
                            ps[slot][:],
                            npk_sb[:, npk_lo:npk_lo + 2, :],
                            ltc[:, off:off + 2, :],
                            start=start, stop=stop, perf_mode=DR)
                    else:
                        nc.tensor.matmul(
                            ps[slot][:], npk_sb[:, npk_lo, :],
                            ltc[:, off, :], start=start, stop=stop)
                elif it[0] == "fir":
                    _, r, stop_fir = it
                    b = r - 1
                    for sdx in (0, 1):
                        for c in (0, 1):
                            src = fsa_sb[:, b + c, :] if sdx == 0 \
                                else fsi_sb[:, b + c, :]
                            nc.tensor.matmul(
                                ps[("r", r)][:], src,
                                fsa_sb[:, 9 + 2 * sdx + c, :],
                                start=False,
                                stop=(stop_fir and sdx == 1 and c == 1))
                elif it[0] == "fin":
                    _, slot = it
                    nc.vector.tensor_scalar_mul(
                        st[:, SLOT15_COL[slot], :], ps[slot][:],
                        1.0 / C_LT)
                elif it[0] == "store":
                    _, lo, hi = it
                    eng = nc.sync if n_store % 2 == 0 else nc.scalar
                    n_store += 1
                    eng.dma_start(
                        out2.ap()[:, lo * P:hi * P],
                        st[:, lo:hi, :].rearrange("p j w -> p (j w)"))
            assert unit_i == len(unit_pos)

    nc.compile()
    return nc


def _build_program(dt_name: str):
    if dt_name == "dr15":
        return _build_program15()
    dr8 = dt_name == "dr8"
    lt_dt_name, op_dt_name = _mode_dtypes(dt_name)
    lt_dt = getattr(mybir.dt, lt_dt_name)
    dt = getattr(mybir.dt, op_dt_name)
    npk_dt = mybir.dt.float8e4 if dr8 else dt
    st_dt = mybir.dt.float16 if dr8 else mybir.dt.float32
    f32 = mybir.dt.float32

    nc = bacc.Bacc("TRN2", target_bir_lowering=False, debug=False,
                   num_devices=N_CORES)

    # all inputs are SBUF images: [128 partitions, free...]; lt is a flat
    # sequence of per-chunk SBUF images in consumption order
    lt = nc.dram_tensor("lt", [TOT_KT * P * P], lt_dt, kind="ExternalInput")
    npk = nc.dram_tensor("npk", [P, S // P, P], npk_dt, kind="ExternalInput")
    fs = nc.dram_tensor("fs", [P, N_SLOTS * 2, P], dt, kind="ExternalInput")
    a2 = nc.dram_tensor("a2", [P, 2, 2, P], dt, kind="ExternalInput")
    # single planar output: row = plane*B + batch (plane 0 = real, 1 = imag)
    # -- matches the psum/staging partition layout, so every store is one
    # full-128-partition DMA instead of two 64-partition ones
    out2 = nc.dram_tensor("out2", [2 * B, N_SLOTS * W], st_dt,
                          kind="ExternalOutput")

    with tile.TileContext(nc) as tc:
        with (
            tc.tile_pool(name="const", bufs=1) as const,
            tc.tile_pool(name="ltp", bufs=9) as ltp,
            tc.tile_pool(name="psum", bufs=1, space=bass.MemorySpace.PSUM) as psum,
            tc.tile_pool(name="stage", bufs=1) as stage,
        ):
            # npk streams in window-sized pieces as the pairs consume it; the
            # first pieces go on the scalar ring so chunk 0 leads the sync
            # ring and the first matmul starts as early as possible.
            npk_sb = const.tile([P, S // P, P], npk_dt)
            nc.scalar.dma_start(npk_sb[:, 0:8, :], npk.ap()[:, 0:8, :])
            nc.scalar.dma_start(npk_sb[:, 8:16, :], npk.ap()[:, 8:16, :])
            fs_sb = const.tile([P, N_SLOTS * 2, P], dt)
            a2_sb = const.tile([P, 2, 2, P], dt)
            fsi_sb = const.tile([P, N_SLOTS * 2, P], dt)

            ps = [psum.tile([P, W], f32, name=f"acc{j}", tag=f"acc{j}")
                  for j in range(N_SLOTS)]
            st = stage.tile([P, 6, W], st_dt)
            stB = stage.tile([P, 2, W], st_dt)
            n_dma = 0
            npk_prefetch = {7: (16, 32), 12: (32, 48), 16: (48, 64)}

            def chunk_dma(n_chunk, ck, flat):
                nonlocal n_dma
                ltc = ltp.tile([P, 16, P], lt_dt, tag="lt", name=f"lt{n_chunk}")
                dma_eng = nc.sync if n_dma % 2 == 0 else nc.scalar
                n_dma += 1
                chunk_inst = dma_eng.dma_start(
                    ltc[:, :ck, :],
                    lt.ap()[flat * P * P:(flat + ck) * P * P].rearrange(
                        "(p n m) -> p n m", p=P, n=ck))
                # fs/a2 aren't needed until the first slots complete at the
                # end of pair 0 -- keep them (and the npk prefetches) behind
                # early chunks with explicit edges so the scheduler can't
                # hoist these dep-free const loads ahead of the chunk stream.
                if n_chunk == 3:
                    fs_inst = nc.sync.dma_start(fs_sb[:], fs.ap())
                    add_dep_helper(fs_inst.ins, chunk_inst.ins, sync=False,
                                   reason="defer fs behind first chunk")
                if n_chunk == 4:
                    a2_inst = dma_eng.dma_start(a2_sb[:], a2.ap())
                    add_dep_helper(a2_inst.ins, chunk_inst.ins, sync=False,
                                   reason="defer a2 behind chunk")
                    # slots complete in ascending order -> derive ascending
                    for g in range(N_SLOTS * 2):
                        nc.vector.tensor_scalar_mul(fsi_sb[:, g, 0:B],
                                                    fs_sb[:, g, B:2 * B], -1.0)
                        nc.vector.tensor_copy(fsi_sb[:, g, B:2 * B],
                                              fs_sb[:, g, 0:B])
                # prefetch the next pair's noise window mid-pair
                if n_chunk in npk_prefetch:
                    lo, hi = npk_prefetch[n_chunk]
                    pf_inst = dma_eng.dma_start(npk_sb[:, lo:hi, :],
                                                npk.ap()[:, lo:hi, :])
                    add_dep_helper(pf_inst.ins, chunk_inst.ins, sync=False,
                                   reason="defer npk prefetch behind chunk")
                return ltc

            def fir_mms(j, stop):
                # FIR: stream A_r against [xr|xi], A_i against [-xi|xr]
                for sdx in (0, 1):
                    for c in (0, 1):
                        g = j * 2 + c
                        src = fs_sb if sdx == 0 else fsi_sb
                        nc.tensor.matmul(ps[j][:], src[:, g, :],
                                         a2_sb[:, sdx, c, :],
                                         start=False,
                                         stop=(stop and sdx == 1 and c == 1))

            def finish_slot(j):
                # slot j's accumulation is complete: evacuate and stream out
                # (dr8: the 1/C_LT that undoes the lt pre-scale rides along)
                dst = st[:, j, :] if j < 6 else stB[:, j - 6, :]
                if dr8:
                    nc.vector.tensor_scalar_mul(dst, ps[j][:], 1.0 / C_LT)
                else:
                    nc.vector.tensor_copy(dst, ps[j][:])


            for n_chunk, (j, kt0, ck, flat) in enumerate(CONSUME):
                ltc = chunk_dma(n_chunk, ck, flat)
                # slots 6/7: their FIR only needs fs/a2, so it runs during
                # pair 2, shortening the serial chain after the last chunk
                fir_early = j >= 6 and kt0 == 32
                last_wins_stop = not (j >= 6)
                if dr8:
                    # DoubleRow: one matmul per PAIR of k-tiles (both
                    # operands fp8e4) at 0.5 cycles/row
                    for i in range(0, ck, 2):
                        is_last = kt0 + ck == SLOT_KT[j] and i == ck - 2
                        nc.tensor.matmul(
                            ps[j][:], npk_sb[:, kt0 + i:kt0 + i + 2, :],
                            ltc[:, i:i + 2, :],
                            start=(kt0 + i == 0),
                            stop=(is_last and not last_wins_stop),
                            perf_mode=mybir.MatmulPerfMode.DoubleRow)
                else:
                    for i in range(ck):
                        is_last = kt0 + ck == SLOT_KT[j] and i == ck - 1
                        nc.tensor.matmul(ps[j][:], npk_sb[:, kt0 + i, :],
                                         ltc[:, i, :],
                                         start=(kt0 + i == 0),
                                         stop=(is_last and not last_wins_stop))
                if fir_early:
                    fir_mms(j, stop=False)
                if kt0 + ck == SLOT_KT[j]:
                    if last_wins_stop:
                        fir_mms(j, stop=True)
                    finish_slot(j)
            # all stores emitted after the load stream so they never steal
            # DMA-engine time from the chunk loads; the first two fire as
            # soon as their copies land (in the loads' natural gaps)
            nc.sync.dma_start(out2.ap()[:, :4 * W],
                              st[:, 0:4].rearrange("p j w -> p (j w)"))
            nc.scalar.dma_start(out2.ap()[:, 4 * W:6 * W],
                                st[:, 4:6].rearrange("p j w -> p (j w)"))
            nc.scalar.dma_start(out2.ap()[:, 7 * W:], stB[:, 1, :])
            nc.sync.dma_start(out2.ap()[:, 6 * W:7 * W], stB[:, 0, :])

    nc.compile()
    return nc


def _sbuf_image(arr_ktpm):
    """[nkt*128, m] k-tile-major -> SBUF image [128, nkt*m]."""
    nktp, m = arr_ktpm.shape
    nkt = nktp // P
    return np.ascontiguousarray(
        arr_ktpm.reshape(nkt, P, m).transpose(1, 0, 2).reshape(P, nkt * m))


def _prep_inputs(x_real, x_imag, a_real, a_imag, L, noise_r, noise_i, N0,
                 dt_name: str):
    mixed8 = dt_name == "mixed8"
    dr8 = dt_name == "dr8"
    a2_scale = np.float32(1.0)
    if dr8:
        # lt holds C_LT*L^T in e4m3; noise is NOT pre-divided (it would land
        # in e4m3's subnormal range) -- instead the taps absorb C_LT and the
        # psum evacuation multiplies everything by 1/C_LT.
        import ml_dtypes
        np_dt = np.float16
        npk_np_dt = ml_dtypes.float8_e4m3
        lt_np_dt = ml_dtypes.float8_e4m3
        lt_scale, npk_scale = np.float32(C_LT), np.float32(1.0)
        a2_scale = np.float32(C_LT)
    elif mixed8:
        import ml_dtypes
        np_dt = np.float16
        npk_np_dt = np_dt
        lt_np_dt = ml_dtypes.float8_e3m4
        lt_scale, npk_scale = np.float32(C_LT), np.float32(1.0 / C_LT)
    else:
        np_dt = _DT_NP[dt_name]
        npk_np_dt = np_dt
        lt_np_dt = np_dt
        lt_scale, npk_scale = np.float32(1.0), np.float32(1.0)

    scale = np.float32(np.sqrt(0.5 * np.power(10.0, np.float64(N0[0]) / 10.0)))

    # packed scaled noise [S, 128]: cols 0:64 real, 64:128 imag
    npk = np.empty((S, 2 * B), np.float32)
    npk[:, :B] = (npk_scale * scale * noise_r).T
    npk[:, B:] = (npk_scale * scale * noise_i).T
    npk = _sbuf_image(npk.astype(npk_np_dt)).reshape(P, S // P, P)

    # x transposed and zero-padded by H on both sides: row r <-> x col r - H
    xpad = np.zeros((S + 2 * H, 2 * B), np.float32)
    xpad[H:H + S, :B] = x_real.T
    xpad[H:H + S, B:] = x_imag.T
    xpad = xpad.astype(np_dt)

    # banded Toeplitz of the taps: A[r, j] = a[j + 2H - r] (valid range only)
    a2 = np.zeros((2, 2 * P, P), np.float32)
    rr = np.arange(2 * P)[:, None]
    jj = np.arange(W)[None, :]
    tap_idx = jj + 2 * H - rr
    valid = (tap_idx >= 0) & (tap_idx < T)
    a2[0][valid] = a2_scale * np.asarray(a_real, np.float32)[tap_idx[valid]]
    a2[1][valid] = a2_scale * np.asarray(a_imag, np.float32)[tap_idx[valid]]
    a2 = _sbuf_image(a2.reshape(2 * 2 * P, P).astype(np_dt)).reshape(P, 2, 2, P)

    L = np.asarray(L, np.float32)

    in_maps = []
    for k in range(N_CORES):
        ltpack = np.zeros((TOT_KT * P * P,), lt_np_dt)
        for j, kt0, ck, flat in CONSUME:
            beta = 8 * j + k
            rows_real = P * (beta + 1)     # non-zero extent in t of strip beta
            r0 = P * kt0                   # this chunk covers t rows r0:r1
            nreal = min(max(rows_real - r0, 0), ck * P)
            if nreal <= 0:
                continue
            block = np.zeros((ck * P, W), lt_np_dt)
            block[:nreal] = np.asarray(
                lt_scale * L[P * beta:P * (beta + 1), r0:r0 + nreal],
                lt_np_dt).T
            img = block.reshape(ck, P, W).transpose(1, 0, 2)
            ltpack[flat * P * P:(flat + ck) * P * P] = img.ravel()

        fsk = np.empty((N_SLOTS * 2, P, 2 * B), np_dt)
        for j in range(N_SLOTS):
            s0 = P * (8 * j + k)           # global first output col of slot
            fsk[j * 2] = xpad[s0:s0 + P]           # [xr | xi] k-tile 0
            fsk[j * 2 + 1] = xpad[s0 + P:s0 + 2 * P]  # k-tile 1
        fsk = _sbuf_image(fsk.reshape(N_SLOTS * 2 * P, 2 * B)).reshape(
            P, N_SLOTS * 2, P)
        in_maps.append({"lt": ltpack, "npk": npk, "fs": fsk, "a2": a2})
    return in_maps


def _prep_inputs15(x_real, x_imag, a_real, a_imag, L, noise_r, noise_i, N0):
    import ml_dtypes
    e4 = ml_dtypes.float8_e4m3
    fp16 = np.float16

    scale = np.float32(np.sqrt(0.5 * np.power(10.0, np.float64(N0[0]) / 10.0)))
    Lf = np.asarray(L, np.float32)

    # global scaled noise [S, 2B] in e4m3 (NOT divided by C_LT -- that would
    # land in e4m3's subnormal range; the taps absorb C_LT instead and the
    # psum evacuation multiplies by 1/C_LT)
    npk_g = np.empty((S, 2 * B), np.float32)
    npk_g[:, :B] = (scale * noise_r).T
    npk_g[:, B:] = (scale * noise_i).T
    npk_g = npk_g.astype(e4)

    # x transposed, offset by H=64: xpad[r] = x[r - 64]
    xpad = np.zeros((8320, 2 * B), np.float32)
    xpad[H:H + S, :B] = x_real.T
    xpad[H:H + S, B:] = x_imag.T
    xpad = xpad.astype(fp16)

    # banded Toeplitz taps, pre-scaled by C_LT: 4 images [128, 128]
    a2 = np.zeros((2, 2 * P, P), np.float32)
    rr = np.arange(2 * P)[:, None]
    jj = np.arange(P)[None, :]
    tap_idx = jj + 2 * H - rr
    valid = (tap_idx >= 0) & (tap_idx < T)
    a2[0][valid] = C_LT * np.asarray(a_real, np.float32)[tap_idx[valid]]
    a2[1][valid] = C_LT * np.asarray(a_imag, np.float32)[tap_idx[valid]]
    a2 = a2.reshape(4, P, P).astype(fp16)

    chunk_sizes, _ = _chunks15()
    in_maps = []
    for k in range(N_CORES):
        # npk locals: [0, 8) = globals [8k, 8k+8); [8, 64) = globals [0, 56)
        npk_loc = np.concatenate(
            [npk_g[1024 * k:1024 * (k + 1)], npk_g[:7168]])
        npk_img = _sbuf_image(npk_loc).reshape(P, S // P, P)

        # lt stream: per-unit blocks in consumption order
        tiles = []
        for it in _stream15():
            if it[0] != "mm":
                continue
            _, slot, npk_lo, sz, _, _ = it
            kind, v = slot
            if kind == "m":
                beta = 8 * v + k
                g0 = npk_lo - 8
            else:
                beta = 8 * k + v - 1
                g0 = 8 * k + npk_lo
            blk = (C_LT * Lf[128 * beta:128 * (beta + 1),
                             128 * g0:128 * (g0 + sz)]).T.astype(e4)
            tiles.append(blk.reshape(sz, P, P))
        tiles = np.concatenate(tiles)
        assert tiles.shape[0] == TOT15
        ltpack = np.empty((TOT15 * P * P,), e4)
        flat = 0
        for ck in chunk_sizes:
            ltpack[flat * P * P:(flat + ck) * P * P] = \
                tiles[flat:flat + ck].transpose(1, 0, 2).ravel()
            flat += ck

        # fsa: 9 x-window images + 4 tap images
        fsa = np.empty((13, P, P), fp16)
        for q in range(9):
            fsa[q] = xpad[1024 * k + 128 * q:1024 * k + 128 * (q + 1)]
        fsa[9:13] = a2
        fsa_img = _sbuf_image(fsa.reshape(13 * P, P)).reshape(P, 13, P)

        in_maps.append({"lt": ltpack, "npk": npk_img, "fsa": fsa_img})
    return in_maps


def _unshard15(results):
    """Add top+bottom partials per strip and reassemble [B, S, 2]."""
    outs = [np.asarray(results[k]["out2"], np.float32)
            for k in range(N_CORES)]
    acc = np.empty((2 * B, S), np.float32)
    for beta in range(64):
        m, r = beta // 8, beta % 8 + 1
        cols = slice(128 * beta, 128 * (beta + 1))
        tc = SLOT15_COL[("r", r)]
        acc[:, cols] = outs[m][:, 128 * tc:128 * (tc + 1)]
        if m >= 1:
            bc = SLOT15_COL[("m", m)]
            acc[:, cols] += outs[beta % 8][:, 128 * bc:128 * (bc + 1)]
    full = np.empty((B, S, 2), np.float32)
    full[:, :, 0] = acc[:B]
    full[:, :, 1] = acc[B:]
    return full


def kernel(x_real, x_imag, a_real, a_imag, L, noise_r, noise_i, N0):
    global LAST_RUN_SECONDS
    inputs = dict(x_real=np.asarray(x_real, np.float32),
                  x_imag=np.asarray(x_imag, np.float32),
                  a_real=np.asarray(a_real, np.float32),
                  a_imag=np.asarray(a_imag, np.float32),
                  L=np.asarray(L, np.float32),
                  noise_r=np.asarray(noise_r, np.float32),
                  noise_i=np.asarray(noise_i, np.float32),
                  N0=np.asarray(N0, np.float32))

    if NOISE_DT not in _CACHE:
        _CACHE[NOISE_DT] = _build_program(NOISE_DT)
    nc = _CACHE[NOISE_DT]

    if NOISE_DT == "dr15":
        in_maps = _prep_inputs15(**inputs)
        t0 = time.time()
        res = run_bass_kernel_spmd(nc, in_maps,
                                   core_ids=list(range(N_CORES)))
        LAST_RUN_SECONDS = time.time() - t0
        return _unshard15(res.results)

    in_maps = _prep_inputs(**inputs, dt_name=NOISE_DT)

    t0 = time.time()
    res = run_bass_kernel_spmd(nc, in_maps, core_ids=list(range(N_CORES)))
    LAST_RUN_SECONDS = time.time() - t0

    planar = np.empty((2, B, N_SLOTS, N_CORES, W), np.float32)
    for k in range(N_CORES):
        o = res.results[k]["out2"].reshape(2, B, N_SLOTS, W)
        planar[0, :, :, k] = o[0]
        planar[1, :, :, k] = o[1]
    full = np.empty((B, S, 2), np.float32)
    full[:, :, 0] = planar[0].reshape(B, S)
    full[:, :, 1] = planar[1].reshape(B, S)
    return full



# revision 67
# speedup vs baseline: 1.8257x; 1.0325x over previous
"""Additive noise channel kernel for 8 Trainium2 NeuronCores.

Computes out[b, s, 0:2] = complex_FIR(x, a)[b, s] + (L @ (scale * noise))[b, s]
with B=64, S=8192, T=129 taps, L lower-triangular [S, S].

Strategy (mode "dr15", the default)
-----------------------------------
The kernel is DMA-bound: the dominant cost is streaming L's nonzero
triangle (32 MB in fp8) through the single modeled 360 GB/s DMA path, so
the design minimizes bytes and keeps the transfer stream gapless.

* DoubleRow fp8: both noise-matmul operands (L^T tiles and the packed
  noise) are fp8e4m3, so every noise matmul runs in DoubleRow perf mode --
  two 128-row k-tiles per instruction at 0.5 cycles/row.  The PE drops to
  ~30% busy and off the critical path; only DMA remains.
* Zero-padding split-strip sharding: output strip beta (extent beta+1
  k-tiles) is written as 8m + r (r in 1..8) and split into a BOTTOM piece
  (k-tiles [0, 8m), pure noise) and a TOP piece (k-tiles [8m, 8m+r), with
  the diagonal + the FIR).  Per core: 7 uniform "m-slots" (sizes 8m) + 8
  uniform "r-slots" (sizes r) = exactly 260 k-tiles -- the 28-tile SPMD
  padding of the (provably minimal) whole-strip staircase is eliminated
  while the instruction stream stays identical across cores.  The two
  partials of a strip land on different cores; the host adds them during
  the unshard it already performs.  npk keeps a duplicated 8-tile per-core
  window (16 KB) so r-slot SBUF addresses are core-invariant.
* Scale folding: lt holds C_LT*L^T (e4m3's sweet spot); the noise is NOT
  pre-divided (that would land subnormal) -- instead the FIR taps absorb
  C_LT and the psum evacuation multiplies by 1/C_LT (a tensor_scalar_mul
  replacing the tensor_copy, so it is free).  Error ~3.1e-3 rel L2, 6x
  under the 2e-2 gate, because fp8 error rides only on the small noise
  term while the dominant FIR stays fp16.
* Schedule: segments run sequentially (m1 r1 m2 r2 ... r7 r8 m7) so npk
  demand grows as its one up-front load streams in, at most ~3 psum banks
  are live (pool cycles 6), and the stream ends with m7's last 4-tile
  chunk: the tail after the final byte is 2 matmuls + one evac + one 32 KB
  store.  fsa (x windows + taps) is pinned behind chunk 0 with a
  sync=False edge; r1/r2's FIR is deferred until it has landed.  ALL
  stores are pinned (sync=False) behind the LAST lt chunk: the load stream
  is gapless, so a store sitting mid-queue would only delay the loads
  behind it -- deferred, the first four stores ride free inside the tail's
  dead time and the loads end at their theoretical minimum (start + load
  bytes, zero slack).  fp16 everywhere except the fp32 PSUM accumulate.
* DMA consolidation: 16 transfers total, every descriptor >= 512 B of
  contiguous run, so HWDGE descriptor-gen (~630 ns per DMA) stays at 45%
  and off the wire's critical path.

Cost-model timeline: 22.4 us/core = 2.0 start + 15.9 gapless loads + 4.5
tail (sem/issue latency constants, with all five stores hidden inside it),
vs 28.1 us for the tuned 8-slot fp8e3 ancestor below.

Legacy modes ("dr8"/"mixed8"/"float16"/"float32", kept for debugging):
the 8-slot staircase -- core k takes strips beta = 8j + k, slot j padded
to 8*(j+1) k-tiles (minimal uniform cover by whole strips, 288 k-tiles),
L^T in fp8e3m4 (mixed8) or fp8e4m3 + DoubleRow (dr8), operands fp16.

On-device everything is TensorE matmuls accumulating in PSUM:
  * noise coloring: lhsT = [scale*noise_r^T | scale*noise_i^T]  (K=128, M=128)
                    rhs  = L^T tile (fp8)                        (K=128, N=128)
    -> psum rows 0:64 = real part, rows 64:128 = imag part; one stream of L
    feeds both real and imag outputs.
  * complex FIR: expressed as x_ext^T @ A where A is the banded Toeplitz
    matrix of the taps, folded into the same PSUM accumulation
    (yr = xr*Ar - xi*Ai, yi = xr*Ai + xi*Ar); the second stationary
    [-xi | xr] is derived on the otherwise-idle VectorE.

Schedule: window-pair-major -- pair p covers k-tiles [16p, 16p+16) of every
still-active slot, so the noise-stationary demand spreads evenly instead of
front-loading; completed slots evacuate + stream out mid-kernel (completing
slots go first within pairs 1-3 so their chains overlap the pair's stream;
pair 0's go last because their FIR needs the late-arriving constants), and
slots 6/7's FIR runs a pair early, so the tail after the last chunk is one
short matmul chain.  The fs/a2/npk constant loads are pinned behind specific
chunks with sync=False dependency edges: without them the Tile scheduler
hoists these dep-free loads ahead of the chunk stream (6.6 us PE stall);
anchored too early they displace pair-0 chunk bytes (1 us PE stall) -- the
swept optimum anchors fs/a2 behind chunks 3/4 and the three noise-window
prefetches behind chunks 7/12/16.

All DRAM inputs are packed host-side in SBUF-image layout (partition-major,
2-4 KB contiguous runs per partition, chunk sequence in exact consumption
order) so the HBM read stream is sequential and every DMA descriptor is
>=1 KB.  Outputs are written planar (real / imag) and interleaved on the
host via one merged planar tensor (row = plane*B + batch, matching the psum
partition layout, so each store is a single full-128-partition DMA).
Cost-model timeline: 28.5 us/core, 0.5 us above the analytic lower bound
for any schedule of this decomposition (max over chunks of arrival time +
remaining PE work, plus the copy/store/sem/barrier constants).
"""

import os
import sys
import time

for _p in ("/opt/trn_rl_repo", "/root/.axon_site/_ro/trn_rl_repo"):
    if _p not in sys.path:
        sys.path.append(_p)

# the bass kernel executes through jax/PJRT on the axon-tunneled NeuronCores
os.environ.setdefault("JAX_PLATFORMS", "axon,cpu")

import numpy as np

import concourse.bass as bass
import concourse.mybir as mybir
import concourse.tile as tile
from concourse.tile import add_dep_helper
from concourse import bacc
from concourse.bass_utils import run_bass_kernel_spmd

B = 64          # batch
S = 8192        # block size
T = 129         # taps
H = (T - 1) // 2  # 64
P = 128         # partitions / k-tile
N_CORES = 8
N_SLOTS = 8     # strips per core
W = 128         # strip width (output columns per slot)
SLOT_KT = [8 * (j + 1) for j in range(N_SLOTS)]   # padded k-tiles per slot
TOT_KT = sum(SLOT_KT)  # 288

# Window-pair-major schedule: pair p covers k-tiles [16p, 16p+16).  All slots
# still alive advance through that window together, so the npk (noise) demand
# spreads evenly across the kernel instead of front-loading, and slots 2p /
# 2p+1 finish in pair p (their outputs stream out mid-kernel).
# CONSUME entries: (slot j, first k-tile kt0, n k-tiles ck, flat offset);
# chunks are laid out back-to-back in DRAM in this (consumption) order.
CONSUME = []
_flat = 0
for _p in range(4):
    # pairs 1-3: completing slots FIRST -- their chunks arrive earliest in
    # the pair, so their FIR + psum evacuation + store all overlap the rest
    # of the pair's chunk stream instead of gating the kernel tail.  Pair 0
    # keeps them LAST: slots 0/1's FIR needs the fs/a2/fsi constants, which
    # only land a few us in.
    if _p == 0:
        _order = list(range(2, N_SLOTS)) + [0, 1]
    elif _p == 3:
        # slot 7 last, with its final chunk split so the chain after the
        # very last byte is only 4 matmuls + one 64 KB store
        _order = [6, 7]
    else:
        _order = [2 * _p, 2 * _p + 1] + list(range(2 * _p + 2, N_SLOTS))
    for _j in _order:
        _ck = 8 if _j == 2 * _p else 16
        CONSUME.append((_j, 16 * _p, _ck, _flat))
        _flat += _ck
assert _flat == TOT_KT
_j9, _kt9, _ck9, _fl9 = CONSUME[-1]
CONSUME[-1:] = [(_j9, _kt9, 12, _fl9), (_j9, _kt9 + 12, 4, _fl9 + 12)]

# Precision mode.  "dr8": L^T AND the noise both in fp8e4m3 so every noise
# matmul runs in DoubleRow perf mode (two k-tiles per instruction, 0.5
# cycles/row); FIR stays fp16; stores fp16.  lt is pre-scaled by C_L=64 (kept
# in e4m3's sweet spot), the noise NOT divided by it; instead the FIR taps
# are pre-scaled by C_L and the psum evacuation multiplies by 1/C_L, which
# costs nothing (tensor_scalar_mul replaces the tensor_copy).
# "mixed8": L^T in fp8e3m4 (pre-scaled by C_LT, folded back via the fp16
# noise stationary), everything else fp16, fp32 PSUM accumulate.
# "float16": all operands fp16 (~3e-4).  "float32": exact (~2e-7), 4x slower.
NOISE_DT = "dr15"

C_LT = 64.0  # fp8 pre-scale: lt stores C_LT*L^T, npk stores scale*noise/C_LT

_DT_NP = {"float32": np.float32, "float16": np.float16}


def _mode_dtypes(dt_name):
    """returns (lt mybir dt, operand mybir dt name) for a mode."""
    if dt_name == "dr8":
        return "float8e4", "float16"
    if dt_name == "mixed8":
        return "float8e3", "float16"
    return dt_name, dt_name

LAST_RUN_SECONDS = None
_CACHE = {}

# ---------------------------------------------------------------------------
# "dr15" mode: zero-padding 15-slot split-strip layout + DoubleRow fp8e4.
#
# Strip beta (0..63, output cols [128b, 128(b+1))) has beta+1 nonzero k-tiles;
# write beta+1 = 8m + r (r in 1..8).  Split it into a BOTTOM piece (k-tiles
# [0, 8m), pure noise partial) and a TOP piece (k-tiles [8m, 8m+r), includes
# the diagonal + the FIR).  Per core: 7 "m-slots" of sizes 8m (m=1..7), one
# per bottom piece of strips {8m + k}, and 8 "r-slots" of sizes r (r=1..8),
# the top pieces of strips {8k + r - 1}.  Total = exactly 260 k-tiles per
# core -- the 28-tile SPMD padding of the 8-slot staircase is gone.  The two
# partials of each strip land on different cores; the host adds them during
# the unshard (it is already gathering anyway).
#
# npk locals: [0, 8) = the per-core window (global k-tiles [8k, 8k+8), used
# by the r-slots, whose global positions are core-dependent), [8, 64) =
# globals [0, 56) (used by the m-slots, core-invariant).  The duplication
# costs 16 KB and buys a uniform instruction stream.
#
# Slots run SEQUENTIALLY (segment-major) in ASCENDING m order with r-slots
# interleaved, so npk demand grows at the pace its pieces stream in, and at
# most ~3 psum accumulations are live at once (PSUM allocates at bank
# granularity: 8 x 2KB; pool cycles 6 bufs).  r1/r2's FIR is deferred until
# after m4 (the fsa/fsi constants only land a few us in); the stream ends
# with m7 whose tail is evacuate + one small store.
# ---------------------------------------------------------------------------
N_SLOT15 = 15
TOT15 = sum(8 * m for m in range(1, 8)) + sum(range(1, 9))  # 260

# completion order -> staging/out2 column
_COMPLETION15 = [("m", 1), ("m", 2), ("m", 3), ("r", 3), ("m", 4), ("r", 1),
                 ("r", 2), ("r", 4), ("m", 5), ("r", 5), ("m", 6), ("r", 6),
                 ("r", 7), ("r", 8), ("m", 7)]
SLOT15_COL = {s: i for i, s in enumerate(_COMPLETION15)}

CHUNK15 = 40  # k-tiles per lt DMA chunk
# npk: leading load [0, NPK_P1_HI) fires before chunk 0; remaining pieces
# [lo, hi) are anchored behind lt chunk index ci (sync=False edges)
NPK_P1_HI = 64
NPK_PIECES15 = {}


def _stream15():
    """Consumption stream: ordered items
      ("mm", slot, npk_local, sz, start, stop)  -- noise matmul unit
      ("fir", r, stop_on_fir)                   -- 4 FIR matmuls for r-slot r
      ("fin", slot)                             -- psum evacuation
      ("store", lo, hi)                         -- staging cols [lo, hi) out
    Noise units consume sz k-tiles of the flat lt stream in order.
    npk locals: m-slot m covers globals [0, 8m) = locals [8, 8+8m);
    r-slot r covers locals [0, r) (the per-core window)."""
    items = []

    def units(kind, v):
        n = 8 * v if kind == "m" else v
        npk0 = 8 if kind == "m" else 0
        lo = 0
        while lo < n:
            sz = 2 if n - lo >= 2 else 1
            stop = kind == "m" and lo + sz == n
            items.append(("mm", (kind, v), npk0 + lo, sz, lo == 0, stop))
            lo += sz

    def fir_fin(r):
        items.append(("fir", r, True))
        items.append(("fin", ("r", r)))

    def mseg(m):
        units("m", m)
        items.append(("fin", ("m", m)))

    mseg(1)
    units("r", 1)
    mseg(2)
    units("r", 2)
    mseg(3)
    units("r", 3)
    fir_fin(3)
    mseg(4)
    fir_fin(1)
    items.append(("store", 0, 4))
    fir_fin(2)
    units("r", 4)
    fir_fin(4)
    mseg(5)
    items.append(("store", 4, 8))
    units("r", 5)
    fir_fin(5)
    mseg(6)
    units("r", 6)
    fir_fin(6)
    units("r", 7)
    fir_fin(7)
    items.append(("store", 8, 12))
    units("r", 8)
    fir_fin(8)
    items.append(("store", 12, 14))
    mseg(7)
    items.append(("store", 14, 15))
    return items


def _chunks15():
    """Split the 260-tile lt stream into DMA chunks at unit boundaries.
    Returns (chunk_sizes, unit_chunk_pos): for each noise unit (in stream
    order) the (chunk_idx, offset) its lt tiles live at."""
    sizes, pos = [], []
    cur = 0
    consumed = 0
    for it in _stream15():
        if it[0] != "mm":
            continue
        sz = it[3]
        # small chunks at the very end keep the post-last-chunk chain short
        cap = CHUNK15 if TOT15 - consumed > 8 else 4
        if cur + sz > cap or not sizes:
            sizes.append(0)
            cur = 0
        pos.append((len(sizes) - 1, cur))
        sizes[-1] += sz
        cur += sz
        consumed += sz
    assert sum(sizes) == TOT15
    return sizes, pos


def _build_program15():
    """15-slot split-strip DoubleRow kernel (mode "dr15")."""
    fp8 = mybir.dt.float8e4
    fp16 = mybir.dt.float16
    f32 = mybir.dt.float32
    DR = mybir.MatmulPerfMode.DoubleRow

    nc = bacc.Bacc("TRN2", target_bir_lowering=False, debug=False,
                   num_devices=N_CORES)

    chunk_sizes, unit_pos = _chunks15()
    n_chunks = len(chunk_sizes)

    lt = nc.dram_tensor("lt", [TOT15 * P * P], fp8, kind="ExternalInput")
    npk = nc.dram_tensor("npk", [P, S // P, P], fp8, kind="ExternalInput")
    # fsa: 9 x-window images (cols 0..8) + 4 tap images (cols 9..12)
    fsa = nc.dram_tensor("fsa", [P, 13, P], fp16, kind="ExternalInput")
    out2 = nc.dram_tensor("out2", [2 * B, N_SLOT15 * P], fp16,
                          kind="ExternalOutput")

    with tile.TileContext(nc) as tc:
        with (
            tc.tile_pool(name="const", bufs=1) as const,
            tc.tile_pool(name="ltp", bufs=6) as ltp,
            tc.tile_pool(name="psum", bufs=6, space=bass.MemorySpace.PSUM) as psum,
            tc.tile_pool(name="stage", bufs=1) as stage,
        ):
            npk_sb = const.tile([P, S // P, P], fp8)
            fsa_sb = const.tile([P, 13, P], fp16)
            fsi_sb = const.tile([P, 9, P], fp16)
            nc.scalar.dma_start(npk_sb[:, 0:NPK_P1_HI, :],
                                npk.ap()[:, 0:NPK_P1_HI, :])

            # psum tiles allocated lazily at first use; same tag -> the pool
            # cycles its 6 bufs in segment order (each reuse is of a slot
            # evacuated several segments earlier, so there is never a stall)
            ps = {}
            st = stage.tile([P, N_SLOT15, P], fp16)

            # chunk DMAs are emitted lazily as the stream consumes them so
            # the Tile scheduler sees them in consumption order
            lt_bufs = {}
            n_dma = 0
            last_chunk_inst = [None]

            def chunk_dma(ci):
                nonlocal n_dma
                ck = chunk_sizes[ci]
                flat = sum(chunk_sizes[:ci])
                ltc = ltp.tile([P, CHUNK15, P], fp8, tag="lt", name=f"lt{ci}")
                dma_eng = nc.sync if n_dma % 2 == 0 else nc.scalar
                n_dma += 1
                inst = dma_eng.dma_start(
                    ltc[:, :ck, :],
                    lt.ap()[flat * P * P:(flat + ck) * P * P].rearrange(
                        "(p n m) -> p n m", p=P, n=ck))
                last_chunk_inst[0] = inst
                # pin const loads behind early chunks so the scheduler can't
                # hoist them ahead of the byte stream
                if ci == 0:
                    fsa_inst = nc.sync.dma_start(fsa_sb[:], fsa.ap())
                    add_dep_helper(fsa_inst.ins, inst.ins, sync=False,
                                   reason="defer fsa behind chunk 0")
                    # derive [-xi | xr] from [xr | xi] in two strided ops
                    nc.vector.tensor_scalar_mul(fsi_sb[:, :, 0:B],
                                                fsa_sb[:, 0:9, B:2 * B], -1.0)
                    nc.vector.tensor_copy(fsi_sb[:, :, B:2 * B],
                                          fsa_sb[:, 0:9, 0:B])
                if ci in NPK_PIECES15:
                    lo, hi = NPK_PIECES15[ci]
                    pp = dma_eng.dma_start(npk_sb[:, lo:hi, :],
                                           npk.ap()[:, lo:hi, :])
                    add_dep_helper(pp.ins, inst.ins, sync=False,
                                   reason="defer npk piece behind chunk")
                return ltc

            n_store = 0
            unit_i = 0
            deferred_stores = []
            for it in _stream15():
                if it[0] == "mm":
                    _, slot, npk_lo, sz, start, stop = it
                    ci, off = unit_pos[unit_i]
                    unit_i += 1
                    if ci not in lt_bufs:
                        lt_bufs[ci] = chunk_dma(ci)
                    ltc = lt_bufs[ci]
                    if slot not in ps:
                        ps[slot] = psum.tile([P, P], f32, tag="ps",
                                             name=f"ps{slot[0]}{slot[1]}")
                    if sz == 2:
                        nc.tensor.matmul(
                            ps[slot][:],
                            npk_sb[:, npk_lo:npk_lo + 2, :],
                            ltc[:, off:off + 2, :],
                            start=start, stop=stop, perf_mode=DR)
                    else:
                        nc.tensor.matmul(
                            ps[slot][:], npk_sb[:, npk_lo, :],
                            ltc[:, off, :], start=start, stop=stop)
                elif it[0] == "fir":
                    _, r, stop_fir = it
                    b = r - 1
                    for sdx in (0, 1):
                        for c in (0, 1):
                            src = fsa_sb[:, b + c, :] if sdx == 0 \
                                else fsi_sb[:, b + c, :]
                            nc.tensor.matmul(
                                ps[("r", r)][:], src,
                                fsa_sb[:, 9 + 2 * sdx + c, :],
                                start=False,
                                stop=(stop_fir and sdx == 1 and c == 1))
                elif it[0] == "fin":
                    _, slot = it
                    nc.vector.tensor_scalar_mul(
                        st[:, SLOT15_COL[slot], :], ps[slot][:],
                        1.0 / C_LT)
                elif it[0] == "store":
                    deferred_stores.append(it)
            # all stores are emitted AFTER the full load stream: the stream
            # is gapless, so a store sitting mid-queue only delays the loads
            # behind it (and with them the whole tail chain)
            for _, lo, hi in deferred_stores:
                eng = nc.sync if n_store % 2 == 0 else nc.scalar
                n_store += 1
                s_inst = eng.dma_start(
                    out2.ap()[:, lo * P:hi * P],
                    st[:, lo:hi, :].rearrange("p j w -> p (j w)"))
                add_dep_helper(s_inst.ins, last_chunk_inst[0].ins, sync=False,
                               reason="stores strictly after the load stream")
            assert unit_i == len(unit_pos)

    nc.compile()
    return nc


def _build_program(dt_name: str):
    if dt_name == "dr15":
        return _build_program15()
    dr8 = dt_name == "dr8"
    lt_dt_name, op_dt_name = _mode_dtypes(dt_name)
    lt_dt = getattr(mybir.dt, lt_dt_name)
    dt = getattr(mybir.dt, op_dt_name)
    npk_dt = mybir.dt.float8e4 if dr8 else dt
    st_dt = mybir.dt.float16 if dr8 else mybir.dt.float32
    f32 = mybir.dt.float32

    nc = bacc.Bacc("TRN2", target_bir_lowering=False, debug=False,
                   num_devices=N_CORES)

    # all inputs are SBUF images: [128 partitions, free...]; lt is a flat
    # sequence of per-chunk SBUF images in consumption order
    lt = nc.dram_tensor("lt", [TOT_KT * P * P], lt_dt, kind="ExternalInput")
    npk = nc.dram_tensor("npk", [P, S // P, P], npk_dt, kind="ExternalInput")
    fs = nc.dram_tensor("fs", [P, N_SLOTS * 2, P], dt, kind="ExternalInput")
    a2 = nc.dram_tensor("a2", [P, 2, 2, P], dt, kind="ExternalInput")
    # single planar output: row = plane*B + batch (plane 0 = real, 1 = imag)
    # -- matches the psum/staging partition layout, so every store is one
    # full-128-partition DMA instead of two 64-partition ones
    out2 = nc.dram_tensor("out2", [2 * B, N_SLOTS * W], st_dt,
                          kind="ExternalOutput")

    with tile.TileContext(nc) as tc:
        with (
            tc.tile_pool(name="const", bufs=1) as const,
            tc.tile_pool(name="ltp", bufs=9) as ltp,
            tc.tile_pool(name="psum", bufs=1, space=bass.MemorySpace.PSUM) as psum,
            tc.tile_pool(name="stage", bufs=1) as stage,
        ):
            # npk streams in window-sized pieces as the pairs consume it; the
            # first pieces go on the scalar ring so chunk 0 leads the sync
            # ring and the first matmul starts as early as possible.
            npk_sb = const.tile([P, S // P, P], npk_dt)
            nc.scalar.dma_start(npk_sb[:, 0:8, :], npk.ap()[:, 0:8, :])
            nc.scalar.dma_start(npk_sb[:, 8:16, :], npk.ap()[:, 8:16, :])
            fs_sb = const.tile([P, N_SLOTS * 2, P], dt)
            a2_sb = const.tile([P, 2, 2, P], dt)
            fsi_sb = const.tile([P, N_SLOTS * 2, P], dt)

            ps = [psum.tile([P, W], f32, name=f"acc{j}", tag=f"acc{j}")
                  for j in range(N_SLOTS)]
            st = stage.tile([P, 6, W], st_dt)
            stB = stage.tile([P, 2, W], st_dt)
            n_dma = 0
            npk_prefetch = {7: (16, 32), 12: (32, 48), 16: (48, 64)}

            def chunk_dma(n_chunk, ck, flat):
                nonlocal n_dma
                ltc = ltp.tile([P, 16, P], lt_dt, tag="lt", name=f"lt{n_chunk}")
                dma_eng = nc.sync if n_dma % 2 == 0 else nc.scalar
                n_dma += 1
                chunk_inst = dma_eng.dma_start(
                    ltc[:, :ck, :],
                    lt.ap()[flat * P * P:(flat + ck) * P * P].rearrange(
                        "(p n m) -> p n m", p=P, n=ck))
                # fs/a2 aren't needed until the first slots complete at the
                # end of pair 0 -- keep them (and the npk prefetches) behind
                # early chunks with explicit edges so the scheduler can't
                # hoist these dep-free const loads ahead of the chunk stream.
                if n_chunk == 3:
                    fs_inst = nc.sync.dma_start(fs_sb[:], fs.ap())
                    add_dep_helper(fs_inst.ins, chunk_inst.ins, sync=False,
                                   reason="defer fs behind first chunk")
                if n_chunk == 4:
                    a2_inst = dma_eng.dma_start(a2_sb[:], a2.ap())
                    add_dep_helper(a2_inst.ins, chunk_inst.ins, sync=False,
                                   reason="defer a2 behind chunk")
                    # slots complete in ascending order -> derive ascending
                    for g in range(N_SLOTS * 2):
                        nc.vector.tensor_scalar_mul(fsi_sb[:, g, 0:B],
                                                    fs_sb[:, g, B:2 * B], -1.0)
                        nc.vector.tensor_copy(fsi_sb[:, g, B:2 * B],
                                              fs_sb[:, g, 0:B])
                # prefetch the next pair's noise window mid-pair
                if n_chunk in npk_prefetch:
                    lo, hi = npk_prefetch[n_chunk]
                    pf_inst = dma_eng.dma_start(npk_sb[:, lo:hi, :],
                                                npk.ap()[:, lo:hi, :])
                    add_dep_helper(pf_inst.ins, chunk_inst.ins, sync=False,
                                   reason="defer npk prefetch behind chunk")
                return ltc

            def fir_mms(j, stop):
                # FIR: stream A_r against [xr|xi], A_i against [-xi|xr]
                for sdx in (0, 1):
                    for c in (0, 1):
                        g = j * 2 + c
                        src = fs_sb if sdx == 0 else fsi_sb
                        nc.tensor.matmul(ps[j][:], src[:, g, :],
                                         a2_sb[:, sdx, c, :],
                                         start=False,
                                         stop=(stop and sdx == 1 and c == 1))

            def finish_slot(j):
                # slot j's accumulation is complete: evacuate and stream out
                # (dr8: the 1/C_LT that undoes the lt pre-scale rides along)
                dst = st[:, j, :] if j < 6 else stB[:, j - 6, :]
                if dr8:
                    nc.vector.tensor_scalar_mul(dst, ps[j][:], 1.0 / C_LT)
                else:
                    nc.vector.tensor_copy(dst, ps[j][:])


            for n_chunk, (j, kt0, ck, flat) in enumerate(CONSUME):
                ltc = chunk_dma(n_chunk, ck, flat)
                # slots 6/7: their FIR only needs fs/a2, so it runs during
                # pair 2, shortening the serial chain after the last chunk
                fir_early = j >= 6 and kt0 == 32
                last_wins_stop = not (j >= 6)
                if dr8:
                    # DoubleRow: one matmul per PAIR of k-tiles (both
                    # operands fp8e4) at 0.5 cycles/row
                    for i in range(0, ck, 2):
                        is_last = kt0 + ck == SLOT_KT[j] and i == ck - 2
                        nc.tensor.matmul(
                            ps[j][:], npk_sb[:, kt0 + i:kt0 + i + 2, :],
                            ltc[:, i:i + 2, :],
                            start=(kt0 + i == 0),
                            stop=(is_last and not last_wins_stop),
                            perf_mode=mybir.MatmulPerfMode.DoubleRow)
                else:
                    for i in range(ck):
                        is_last = kt0 + ck == SLOT_KT[j] and i == ck - 1
                        nc.tensor.matmul(ps[j][:], npk_sb[:, kt0 + i, :],
                                         ltc[:, i, :],
                                         start=(kt0 + i == 0),
                                         stop=(is_last and not last_wins_stop))
                if fir_early:
                    fir_mms(j, stop=False)
                if kt0 + ck == SLOT_KT[j]:
                    if last_wins_stop:
                        fir_mms(j, stop=True)
                    finish_slot(j)
            # all stores emitted after the load stream so they never steal
            # DMA-engine time from the chunk loads; the first two fire as
            # soon as their copies land (in the loads' natural gaps)
            nc.sync.dma_start(out2.ap()[:, :4 * W],
                              st[:, 0:4].rearrange("p j w -> p (j w)"))
            nc.scalar.dma_start(out2.ap()[:, 4 * W:6 * W],
                                st[:, 4:6].rearrange("p j w -> p (j w)"))
            nc.scalar.dma_start(out2.ap()[:, 7 * W:], stB[:, 1, :])
            nc.sync.dma_start(out2.ap()[:, 6 * W:7 * W], stB[:, 0, :])

    nc.compile()
    return nc


def _sbuf_image(arr_ktpm):
    """[nkt*128, m] k-tile-major -> SBUF image [128, nkt*m]."""
    nktp, m = arr_ktpm.shape
    nkt = nktp // P
    return np.ascontiguousarray(
        arr_ktpm.reshape(nkt, P, m).transpose(1, 0, 2).reshape(P, nkt * m))


def _prep_inputs(x_real, x_imag, a_real, a_imag, L, noise_r, noise_i, N0,
                 dt_name: str):
    mixed8 = dt_name == "mixed8"
    dr8 = dt_name == "dr8"
    a2_scale = np.float32(1.0)
    if dr8:
        # lt holds C_LT*L^T in e4m3; noise is NOT pre-divided (it would land
        # in e4m3's subnormal range) -- instead the taps absorb C_LT and the
        # psum evacuation multiplies everything by 1/C_LT.
        import ml_dtypes
        np_dt = np.float16
        npk_np_dt = ml_dtypes.float8_e4m3
        lt_np_dt = ml_dtypes.float8_e4m3
        lt_scale, npk_scale = np.float32(C_LT), np.float32(1.0)
        a2_scale = np.float32(C_LT)
    elif mixed8:
        import ml_dtypes
        np_dt = np.float16
        npk_np_dt = np_dt
        lt_np_dt = ml_dtypes.float8_e3m4
        lt_scale, npk_scale = np.float32(C_LT), np.float32(1.0 / C_LT)
    else:
        np_dt = _DT_NP[dt_name]
        npk_np_dt = np_dt
        lt_np_dt = np_dt
        lt_scale, npk_scale = np.float32(1.0), np.float32(1.0)

    scale = np.float32(np.sqrt(0.5 * np.power(10.0, np.float64(N0[0]) / 10.0)))

    # packed scaled noise [S, 128]: cols 0:64 real, 64:128 imag
    npk = np.empty((S, 2 * B), np.float32)
    npk[:, :B] = (npk_scale * scale * noise_r).T
    npk[:, B:] = (npk_scale * scale * noise_i).T
    npk = _sbuf_image(npk.astype(npk_np_dt)).reshape(P, S // P, P)

    # x transposed and zero-padded by H on both sides: row r <-> x col r - H
    xpad = np.zeros((S + 2 * H, 2 * B), np.float32)
    xpad[H:H + S, :B] = x_real.T
    xpad[H:H + S, B:] = x_imag.T
    xpad = xpad.astype(np_dt)

    # banded Toeplitz of the taps: A[r, j] = a[j + 2H - r] (valid range only)
    a2 = np.zeros((2, 2 * P, P), np.float32)
    rr = np.arange(2 * P)[:, None]
    jj = np.arange(W)[None, :]
    tap_idx = jj + 2 * H - rr
    valid = (tap_idx >= 0) & (tap_idx < T)
    a2[0][valid] = a2_scale * np.asarray(a_real, np.float32)[tap_idx[valid]]
    a2[1][valid] = a2_scale * np.asarray(a_imag, np.float32)[tap_idx[valid]]
    a2 = _sbuf_image(a2.reshape(2 * 2 * P, P).astype(np_dt)).reshape(P, 2, 2, P)

    L = np.asarray(L, np.float32)

    in_maps = []
    for k in range(N_CORES):
        ltpack = np.zeros((TOT_KT * P * P,), lt_np_dt)
        for j, kt0, ck, flat in CONSUME:
            beta = 8 * j + k
            rows_real = P * (beta + 1)     # non-zero extent in t of strip beta
            r0 = P * kt0                   # this chunk covers t rows r0:r1
            nreal = min(max(rows_real - r0, 0), ck * P)
            if nreal <= 0:
                continue
            block = np.zeros((ck * P, W), lt_np_dt)
            block[:nreal] = np.asarray(
                lt_scale * L[P * beta:P * (beta + 1), r0:r0 + nreal],
                lt_np_dt).T
            img = block.reshape(ck, P, W).transpose(1, 0, 2)
            ltpack[flat * P * P:(flat + ck) * P * P] = img.ravel()

        fsk = np.empty((N_SLOTS * 2, P, 2 * B), np_dt)
        for j in range(N_SLOTS):
            s0 = P * (8 * j + k)           # global first output col of slot
            fsk[j * 2] = xpad[s0:s0 + P]           # [xr | xi] k-tile 0
            fsk[j * 2 + 1] = xpad[s0 + P:s0 + 2 * P]  # k-tile 1
        fsk = _sbuf_image(fsk.reshape(N_SLOTS * 2 * P, 2 * B)).reshape(
            P, N_SLOTS * 2, P)
        in_maps.append({"lt": ltpack, "npk": npk, "fs": fsk, "a2": a2})
    return in_maps


def _prep_inputs15(x_real, x_imag, a_real, a_imag, L, noise_r, noise_i, N0):
    import ml_dtypes
    e4 = ml_dtypes.float8_e4m3
    fp16 = np.float16

    scale = np.float32(np.sqrt(0.5 * np.power(10.0, np.float64(N0[0]) / 10.0)))
    Lf = np.asarray(L, np.float32)

    # global scaled noise [S, 2B] in e4m3 (NOT divided by C_LT -- that would
    # land in e4m3's subnormal range; the taps absorb C_LT instead and the
    # psum evacuation multiplies by 1/C_LT)
    npk_g = np.empty((S, 2 * B), np.float32)
    npk_g[:, :B] = (scale * noise_r).T
    npk_g[:, B:] = (scale * noise_i).T
    npk_g = npk_g.astype(e4)

    # x transposed, offset by H=64: xpad[r] = x[r - 64]
    xpad = np.zeros((8320, 2 * B), np.float32)
    xpad[H:H + S, :B] = x_real.T
    xpad[H:H + S, B:] = x_imag.T
    xpad = xpad.astype(fp16)

    # banded Toeplitz taps, pre-scaled by C_LT: 4 images [128, 128]
    a2 = np.zeros((2, 2 * P, P), np.float32)
    rr = np.arange(2 * P)[:, None]
    jj = np.arange(P)[None, :]
    tap_idx = jj + 2 * H - rr
    valid = (tap_idx >= 0) & (tap_idx < T)
    a2[0][valid] = C_LT * np.asarray(a_real, np.float32)[tap_idx[valid]]
    a2[1][valid] = C_LT * np.asarray(a_imag, np.float32)[tap_idx[valid]]
    a2 = a2.reshape(4, P, P).astype(fp16)

    chunk_sizes, _ = _chunks15()
    in_maps = []
    for k in range(N_CORES):
        # npk locals: [0, 8) = globals [8k, 8k+8); [8, 64) = globals [0, 56)
        npk_loc = np.concatenate(
            [npk_g[1024 * k:1024 * (k + 1)], npk_g[:7168]])
        npk_img = _sbuf_image(npk_loc).reshape(P, S // P, P)

        # lt stream: per-unit blocks in consumption order
        tiles = []
        for it in _stream15():
            if it[0] != "mm":
                continue
            _, slot, npk_lo, sz, _, _ = it
            kind, v = slot
            if kind == "m":
                beta = 8 * v + k
                g0 = npk_lo - 8
            else:
                beta = 8 * k + v - 1
                g0 = 8 * k + npk_lo
            blk = (C_LT * Lf[128 * beta:128 * (beta + 1),
                             128 * g0:128 * (g0 + sz)]).T.astype(e4)
            tiles.append(blk.reshape(sz, P, P))
        tiles = np.concatenate(tiles)
        assert tiles.shape[0] == TOT15
        ltpack = np.empty((TOT15 * P * P,), e4)
        flat = 0
        for ck in chunk_sizes:
            ltpack[flat * P * P:(flat + ck) * P * P] = \
                tiles[flat:flat + ck].transpose(1, 0, 2).ravel()
            flat += ck

        # fsa: 9 x-window images + 4 tap images
        fsa = np.empty((13, P, P), fp16)
        for q in range(9):
            fsa[q] = xpad[1024 * k + 128 * q:1024 * k + 128 * (q + 1)]
        fsa[9:13] = a2
        fsa_img = _sbuf_image(fsa.reshape(13 * P, P)).reshape(P, 13, P)

        in_maps.append({"lt": ltpack, "npk": npk_img, "fsa": fsa_img})
    return in_maps


def _unshard15(results):
    """Add top+bottom partials per strip and reassemble [B, S, 2]."""
    outs = [np.asarray(results[k]["out2"], np.float32)
            for k in range(N_CORES)]
    acc = np.empty((2 * B, S), np.float32)
    for beta in range(64):
        m, r = beta // 8, beta % 8 + 1
        cols = slice(128 * beta, 128 * (beta + 1))
        tc = SLOT15_COL[("r", r)]
        acc[:, cols] = outs[m][:, 128 * tc:128 * (tc + 1)]
        if m >= 1:
            bc = SLOT15_COL[("m", m)]
            acc[:, cols] += outs[beta % 8][:, 128 * bc:128 * (bc + 1)]
    full = np.empty((B, S, 2), np.float32)
    full[:, :, 0] = acc[:B]
    full[:, :, 1] = acc[B:]
    return full


def kernel(x_real, x_imag, a_real, a_imag, L, noise_r, noise_i, N0):
    global LAST_RUN_SECONDS
    inputs = dict(x_real=np.asarray(x_real, np.float32),
                  x_imag=np.asarray(x_imag, np.float32),
                  a_real=np.asarray(a_real, np.float32),
                  a_imag=np.asarray(a_imag, np.float32),
                  L=np.asarray(L, np.float32),
                  noise_r=np.asarray(noise_r, np.float32),
                  noise_i=np.asarray(noise_i, np.float32),
                  N0=np.asarray(N0, np.float32))

    if NOISE_DT not in _CACHE:
        _CACHE[NOISE_DT] = _build_program(NOISE_DT)
    nc = _CACHE[NOISE_DT]

    if NOISE_DT == "dr15":
        in_maps = _prep_inputs15(**inputs)
        t0 = time.time()
        res = run_bass_kernel_spmd(nc, in_maps,
                                   core_ids=list(range(N_CORES)))
        LAST_RUN_SECONDS = time.time() - t0
        return _unshard15(res.results)

    in_maps = _prep_inputs(**inputs, dt_name=NOISE_DT)

    t0 = time.time()
    res = run_bass_kernel_spmd(nc, in_maps, core_ids=list(range(N_CORES)))
    LAST_RUN_SECONDS = time.time() - t0

    planar = np.empty((2, B, N_SLOTS, N_CORES, W), np.float32)
    for k in range(N_CORES):
        o = res.results[k]["out2"].reshape(2, B, N_SLOTS, W)
        planar[0, :, :, k] = o[0]
        planar[1, :, :, k] = o[1]
    full = np.empty((B, S, 2), np.float32)
    full[:, :, 0] = planar[0].reshape(B, S)
    full[:, :, 1] = planar[1].reshape(B, S)
    return full



# revision 68
# speedup vs baseline: 1.8331x; 1.0041x over previous
"""Additive noise channel kernel for 8 Trainium2 NeuronCores.

Computes out[b, s, 0:2] = complex_FIR(x, a)[b, s] + (L @ (scale * noise))[b, s]
with B=64, S=8192, T=129 taps, L lower-triangular [S, S].

Strategy (mode "dr15", the default)
-----------------------------------
The kernel is DMA-bound: the dominant cost is streaming L's nonzero
triangle (32 MB in fp8) through the single modeled 360 GB/s DMA path, so
the design minimizes bytes and keeps the transfer stream gapless.

* DoubleRow fp8: both noise-matmul operands (L^T tiles and the packed
  noise) are fp8e4m3, so every noise matmul runs in DoubleRow perf mode --
  two 128-row k-tiles per instruction at 0.5 cycles/row.  The PE drops to
  ~30% busy and off the critical path; only DMA remains.
* Zero-padding split-strip sharding: output strip beta (extent beta+1
  k-tiles) is written as 8m + r (r in 1..8) and split into a BOTTOM piece
  (k-tiles [0, 8m), pure noise) and a TOP piece (k-tiles [8m, 8m+r), with
  the diagonal + the FIR).  Per core: 7 uniform "m-slots" (sizes 8m) + 8
  uniform "r-slots" (sizes r) = exactly 260 k-tiles -- the 28-tile SPMD
  padding of the (provably minimal) whole-strip staircase is eliminated
  while the instruction stream stays identical across cores.  The two
  partials of a strip land on different cores; the host adds them during
  the unshard it already performs.  npk keeps a duplicated 8-tile per-core
  window (16 KB) so r-slot SBUF addresses are core-invariant.
* Scale folding: lt holds C_LT*L^T (e4m3's sweet spot); the noise is NOT
  pre-divided (that would land subnormal) -- instead the FIR taps absorb
  C_LT and the psum evacuation multiplies by 1/C_LT (a tensor_scalar_mul
  replacing the tensor_copy, so it is free).  Error ~3.1e-3 rel L2, 6x
  under the 2e-2 gate, because fp8 error rides only on the small noise
  term while the dominant FIR stays fp16.
* Schedule: segments run sequentially (m1 r1 m2 r2 ... r7 r8 m7) so npk
  demand grows as its one up-front load streams in, at most ~3 psum banks
  are live (pool cycles 6), and the stream ends with m7's last 4-tile
  chunk: the tail after the final byte is 2 matmuls + one evac + one 32 KB
  store.  fsa (x windows + taps) is pinned behind chunk 0 with a
  sync=False edge; r1/r2's FIR is deferred until it has landed.  ALL
  stores are pinned (sync=False) behind the LAST lt chunk: the load stream
  is gapless, so a store sitting mid-queue would only delay the loads
  behind it -- deferred, the first four stores ride free inside the tail's
  dead time and the loads end at their theoretical minimum (start + load
  bytes, zero slack).  fp16 everywhere except the fp32 PSUM accumulate.
* DMA consolidation: 16 transfers total, every descriptor >= 512 B of
  contiguous run, so HWDGE descriptor-gen (~630 ns per DMA) stays at 45%
  and off the wire's critical path.

Cost-model timeline: 22.4 us/core = 2.0 start + 15.9 gapless loads + 4.5
tail (sem/issue latency constants, with all five stores hidden inside it),
vs 28.1 us for the tuned 8-slot fp8e3 ancestor below.

Legacy modes ("dr8"/"mixed8"/"float16"/"float32", kept for debugging):
the 8-slot staircase -- core k takes strips beta = 8j + k, slot j padded
to 8*(j+1) k-tiles (minimal uniform cover by whole strips, 288 k-tiles),
L^T in fp8e3m4 (mixed8) or fp8e4m3 + DoubleRow (dr8), operands fp16.

On-device everything is TensorE matmuls accumulating in PSUM:
  * noise coloring: lhsT = [scale*noise_r^T | scale*noise_i^T]  (K=128, M=128)
                    rhs  = L^T tile (fp8)                        (K=128, N=128)
    -> psum rows 0:64 = real part, rows 64:128 = imag part; one stream of L
    feeds both real and imag outputs.
  * complex FIR: expressed as x_ext^T @ A where A is the banded Toeplitz
    matrix of the taps, folded into the same PSUM accumulation
    (yr = xr*Ar - xi*Ai, yi = xr*Ai + xi*Ar); the second stationary
    [-xi | xr] is derived on the otherwise-idle VectorE.

Schedule: window-pair-major -- pair p covers k-tiles [16p, 16p+16) of every
still-active slot, so the noise-stationary demand spreads evenly instead of
front-loading; completed slots evacuate + stream out mid-kernel (completing
slots go first within pairs 1-3 so their chains overlap the pair's stream;
pair 0's go last because their FIR needs the late-arriving constants), and
slots 6/7's FIR runs a pair early, so the tail after the last chunk is one
short matmul chain.  The fs/a2/npk constant loads are pinned behind specific
chunks with sync=False dependency edges: without them the Tile scheduler
hoists these dep-free loads ahead of the chunk stream (6.6 us PE stall);
anchored too early they displace pair-0 chunk bytes (1 us PE stall) -- the
swept optimum anchors fs/a2 behind chunks 3/4 and the three noise-window
prefetches behind chunks 7/12/16.

All DRAM inputs are packed host-side in SBUF-image layout (partition-major,
2-4 KB contiguous runs per partition, chunk sequence in exact consumption
order) so the HBM read stream is sequential and every DMA descriptor is
>=1 KB.  Outputs are written planar (real / imag) and interleaved on the
host via one merged planar tensor (row = plane*B + batch, matching the psum
partition layout, so each store is a single full-128-partition DMA).
Cost-model timeline: 28.5 us/core, 0.5 us above the analytic lower bound
for any schedule of this decomposition (max over chunks of arrival time +
remaining PE work, plus the copy/store/sem/barrier constants).
"""

import os
import sys
import time

for _p in ("/opt/trn_rl_repo", "/root/.axon_site/_ro/trn_rl_repo"):
    if _p not in sys.path:
        sys.path.append(_p)

# the bass kernel executes through jax/PJRT on the axon-tunneled NeuronCores
os.environ.setdefault("JAX_PLATFORMS", "axon,cpu")

import numpy as np

import concourse.bass as bass
import concourse.mybir as mybir
import concourse.tile as tile
from concourse.tile import add_dep_helper
from concourse import bacc
from concourse.bass_utils import run_bass_kernel_spmd

B = 64          # batch
S = 8192        # block size
T = 129         # taps
H = (T - 1) // 2  # 64
P = 128         # partitions / k-tile
N_CORES = 8
N_SLOTS = 8     # strips per core
W = 128         # strip width (output columns per slot)
SLOT_KT = [8 * (j + 1) for j in range(N_SLOTS)]   # padded k-tiles per slot
TOT_KT = sum(SLOT_KT)  # 288

# Window-pair-major schedule: pair p covers k-tiles [16p, 16p+16).  All slots
# still alive advance through that window together, so the npk (noise) demand
# spreads evenly across the kernel instead of front-loading, and slots 2p /
# 2p+1 finish in pair p (their outputs stream out mid-kernel).
# CONSUME entries: (slot j, first k-tile kt0, n k-tiles ck, flat offset);
# chunks are laid out back-to-back in DRAM in this (consumption) order.
CONSUME = []
_flat = 0
for _p in range(4):
    # pairs 1-3: completing slots FIRST -- their chunks arrive earliest in
    # the pair, so their FIR + psum evacuation + store all overlap the rest
    # of the pair's chunk stream instead of gating the kernel tail.  Pair 0
    # keeps them LAST: slots 0/1's FIR needs the fs/a2/fsi constants, which
    # only land a few us in.
    if _p == 0:
        _order = list(range(2, N_SLOTS)) + [0, 1]
    elif _p == 3:
        # slot 7 last, with its final chunk split so the chain after the
        # very last byte is only 4 matmuls + one 64 KB store
        _order = [6, 7]
    else:
        _order = [2 * _p, 2 * _p + 1] + list(range(2 * _p + 2, N_SLOTS))
    for _j in _order:
        _ck = 8 if _j == 2 * _p else 16
        CONSUME.append((_j, 16 * _p, _ck, _flat))
        _flat += _ck
assert _flat == TOT_KT
_j9, _kt9, _ck9, _fl9 = CONSUME[-1]
CONSUME[-1:] = [(_j9, _kt9, 12, _fl9), (_j9, _kt9 + 12, 4, _fl9 + 12)]

# Precision mode.  "dr8": L^T AND the noise both in fp8e4m3 so every noise
# matmul runs in DoubleRow perf mode (two k-tiles per instruction, 0.5
# cycles/row); FIR stays fp16; stores fp16.  lt is pre-scaled by C_L=64 (kept
# in e4m3's sweet spot), the noise NOT divided by it; instead the FIR taps
# are pre-scaled by C_L and the psum evacuation multiplies by 1/C_L, which
# costs nothing (tensor_scalar_mul replaces the tensor_copy).
# "mixed8": L^T in fp8e3m4 (pre-scaled by C_LT, folded back via the fp16
# noise stationary), everything else fp16, fp32 PSUM accumulate.
# "float16": all operands fp16 (~3e-4).  "float32": exact (~2e-7), 4x slower.
NOISE_DT = "dr15"

C_LT = 64.0  # fp8 pre-scale: lt stores C_LT*L^T, npk stores scale*noise/C_LT

_DT_NP = {"float32": np.float32, "float16": np.float16}


def _mode_dtypes(dt_name):
    """returns (lt mybir dt, operand mybir dt name) for a mode."""
    if dt_name == "dr8":
        return "float8e4", "float16"
    if dt_name == "mixed8":
        return "float8e3", "float16"
    return dt_name, dt_name

LAST_RUN_SECONDS = None
_CACHE = {}

# ---------------------------------------------------------------------------
# "dr15" mode: zero-padding 15-slot split-strip layout + DoubleRow fp8e4.
#
# Strip beta (0..63, output cols [128b, 128(b+1))) has beta+1 nonzero k-tiles;
# write beta+1 = 8m + r (r in 1..8).  Split it into a BOTTOM piece (k-tiles
# [0, 8m), pure noise partial) and a TOP piece (k-tiles [8m, 8m+r), includes
# the diagonal + the FIR).  Per core: 7 "m-slots" of sizes 8m (m=1..7), one
# per bottom piece of strips {8m + k}, and 8 "r-slots" of sizes r (r=1..8),
# the top pieces of strips {8k + r - 1}.  Total = exactly 260 k-tiles per
# core -- the 28-tile SPMD padding of the 8-slot staircase is gone.  The two
# partials of each strip land on different cores; the host adds them during
# the unshard (it is already gathering anyway).
#
# npk locals: [0, 8) = the per-core window (global k-tiles [8k, 8k+8), used
# by the r-slots, whose global positions are core-dependent), [8, 64) =
# globals [0, 56) (used by the m-slots, core-invariant).  The duplication
# costs 16 KB and buys a uniform instruction stream.
#
# Slots run SEQUENTIALLY (segment-major) in ASCENDING m order with r-slots
# interleaved, so npk demand grows at the pace its pieces stream in, and at
# most ~3 psum accumulations are live at once (PSUM allocates at bank
# granularity: 8 x 2KB; pool cycles 6 bufs).  r1/r2's FIR is deferred until
# after m4 (the fsa/fsi constants only land a few us in); the stream ends
# with m7 whose tail is evacuate + one small store.
# ---------------------------------------------------------------------------
N_SLOT15 = 15
TOT15 = sum(8 * m for m in range(1, 8)) + sum(range(1, 9))  # 260

# completion order -> staging/out2 column
_COMPLETION15 = [("m", 1), ("m", 2), ("m", 3), ("r", 3), ("m", 4), ("r", 1),
                 ("r", 2), ("r", 4), ("m", 5), ("r", 5), ("m", 6), ("r", 6),
                 ("r", 7), ("r", 8), ("m", 7)]
SLOT15_COL = {s: i for i, s in enumerate(_COMPLETION15)}

CHUNK15 = 40  # k-tiles per lt DMA chunk
# npk: leading load [0, NPK_P1_HI) fires before chunk 0; remaining pieces
# [lo, hi) are anchored behind lt chunk index ci (sync=False edges)
NPK_P1_HI = 64
NPK_PIECES15 = {}


def _stream15():
    """Consumption stream: ordered items
      ("mm", slot, npk_local, sz, start, stop)  -- noise matmul unit
      ("fir", r, stop_on_fir)                   -- 4 FIR matmuls for r-slot r
      ("fin", slot)                             -- psum evacuation
      ("store", lo, hi)                         -- staging cols [lo, hi) out
    Noise units consume sz k-tiles of the flat lt stream in order.
    npk locals: m-slot m covers globals [0, 8m) = locals [8, 8+8m);
    r-slot r covers locals [0, r) (the per-core window)."""
    items = []

    def units(kind, v):
        n = 8 * v if kind == "m" else v
        npk0 = 8 if kind == "m" else 0
        lo = 0
        while lo < n:
            sz = 2 if n - lo >= 2 else 1
            stop = kind == "m" and lo + sz == n
            items.append(("mm", (kind, v), npk0 + lo, sz, lo == 0, stop))
            lo += sz

    def fir_fin(r):
        items.append(("fir", r, True))
        items.append(("fin", ("r", r)))

    def mseg(m):
        units("m", m)
        items.append(("fin", ("m", m)))

    mseg(1)
    units("r", 1)
    mseg(2)
    units("r", 2)
    mseg(3)
    units("r", 3)
    fir_fin(3)
    mseg(4)
    fir_fin(1)
    items.append(("store", 0, 4))
    fir_fin(2)
    units("r", 4)
    fir_fin(4)
    mseg(5)
    items.append(("store", 4, 8))
    units("r", 5)
    fir_fin(5)
    mseg(6)
    units("r", 6)
    fir_fin(6)
    units("r", 7)
    fir_fin(7)
    items.append(("store", 8, 12))
    units("r", 8)
    fir_fin(8)
    items.append(("store", 12, 14))
    mseg(7)
    items.append(("store", 14, 15))
    return items


def _chunks15():
    """Split the 260-tile lt stream into DMA chunks at unit boundaries.
    Returns (chunk_sizes, unit_chunk_pos): for each noise unit (in stream
    order) the (chunk_idx, offset) its lt tiles live at."""
    sizes, pos = [], []
    cur = 0
    consumed = 0
    for it in _stream15():
        if it[0] != "mm":
            continue
        sz = it[3]
        # small chunks at the very end keep the post-last-chunk chain short
        cap = CHUNK15 if TOT15 - consumed > 8 else 4
        if cur + sz > cap or not sizes:
            sizes.append(0)
            cur = 0
        pos.append((len(sizes) - 1, cur))
        sizes[-1] += sz
        cur += sz
        consumed += sz
    assert sum(sizes) == TOT15
    return sizes, pos


def _build_program15():
    """15-slot split-strip DoubleRow kernel (mode "dr15")."""
    fp8 = mybir.dt.float8e4
    fp16 = mybir.dt.float16
    f32 = mybir.dt.float32
    DR = mybir.MatmulPerfMode.DoubleRow

    nc = bacc.Bacc("TRN2", target_bir_lowering=False, debug=False,
                   num_devices=N_CORES)

    chunk_sizes, unit_pos = _chunks15()
    n_chunks = len(chunk_sizes)

    lt = nc.dram_tensor("lt", [TOT15 * P * P], fp8, kind="ExternalInput")
    npk = nc.dram_tensor("npk", [P, S // P, P], fp8, kind="ExternalInput")
    # fsa: 9 x-window images (cols 0..8) + 4 tap images (cols 9..12)
    fsa = nc.dram_tensor("fsa", [P, 13, P], fp16, kind="ExternalInput")
    out2 = nc.dram_tensor("out2", [2 * B, (N_SLOT15 - 1) * P], fp16,
                          kind="ExternalOutput")
    # m7 (the tail column, pure noise partials) stages and stores in fp8e4:
    # halves the one store transfer that sits on the critical tail chain
    outm7 = nc.dram_tensor("outm7", [2 * B, P], fp8, kind="ExternalOutput")

    with tile.TileContext(nc) as tc:
        with (
            tc.tile_pool(name="const", bufs=1) as const,
            tc.tile_pool(name="ltp", bufs=6) as ltp,
            tc.tile_pool(name="psum", bufs=6, space=bass.MemorySpace.PSUM) as psum,
            tc.tile_pool(name="stage", bufs=1) as stage,
        ):
            npk_sb = const.tile([P, S // P, P], fp8)
            fsa_sb = const.tile([P, 13, P], fp16)
            fsi_sb = const.tile([P, 9, P], fp16)
            nc.scalar.dma_start(npk_sb[:, 0:NPK_P1_HI, :],
                                npk.ap()[:, 0:NPK_P1_HI, :])

            # psum tiles allocated lazily at first use; same tag -> the pool
            # cycles its 6 bufs in segment order (each reuse is of a slot
            # evacuated several segments earlier, so there is never a stall)
            ps = {}
            st = stage.tile([P, N_SLOT15 - 1, P], fp16)
            st8 = stage.tile([P, P], fp8)

            # chunk DMAs are emitted lazily as the stream consumes them so
            # the Tile scheduler sees them in consumption order
            lt_bufs = {}
            n_dma = 0
            last_chunk_inst = [None]

            def chunk_dma(ci):
                nonlocal n_dma
                ck = chunk_sizes[ci]
                flat = sum(chunk_sizes[:ci])
                ltc = ltp.tile([P, CHUNK15, P], fp8, tag="lt", name=f"lt{ci}")
                dma_eng = nc.sync if n_dma % 2 == 0 else nc.scalar
                n_dma += 1
                inst = dma_eng.dma_start(
                    ltc[:, :ck, :],
                    lt.ap()[flat * P * P:(flat + ck) * P * P].rearrange(
                        "(p n m) -> p n m", p=P, n=ck))
                last_chunk_inst[0] = inst
                # pin const loads behind early chunks so the scheduler can't
                # hoist them ahead of the byte stream
                if ci == 0:
                    fsa_inst = nc.sync.dma_start(fsa_sb[:], fsa.ap())
                    add_dep_helper(fsa_inst.ins, inst.ins, sync=False,
                                   reason="defer fsa behind chunk 0")
                    # derive [-xi | xr] from [xr | xi] in two strided ops
                    nc.vector.tensor_scalar_mul(fsi_sb[:, :, 0:B],
                                                fsa_sb[:, 0:9, B:2 * B], -1.0)
                    nc.vector.tensor_copy(fsi_sb[:, :, B:2 * B],
                                          fsa_sb[:, 0:9, 0:B])
                if ci in NPK_PIECES15:
                    lo, hi = NPK_PIECES15[ci]
                    pp = dma_eng.dma_start(npk_sb[:, lo:hi, :],
                                           npk.ap()[:, lo:hi, :])
                    add_dep_helper(pp.ins, inst.ins, sync=False,
                                   reason="defer npk piece behind chunk")
                return ltc

            n_store = 0
            unit_i = 0
            deferred_stores = []
            for it in _stream15():
                if it[0] == "mm":
                    _, slot, npk_lo, sz, start, stop = it
                    ci, off = unit_pos[unit_i]
                    unit_i += 1
                    if ci not in lt_bufs:
                        lt_bufs[ci] = chunk_dma(ci)
                    ltc = lt_bufs[ci]
                    if slot not in ps:
                        ps[slot] = psum.tile([P, P], f32, tag="ps",
                                             name=f"ps{slot[0]}{slot[1]}")
                    if sz == 2:
                        nc.tensor.matmul(
                            ps[slot][:],
                            npk_sb[:, npk_lo:npk_lo + 2, :],
                            ltc[:, off:off + 2, :],
                            start=start, stop=stop, perf_mode=DR)
                    else:
                        nc.tensor.matmul(
                            ps[slot][:], npk_sb[:, npk_lo, :],
                            ltc[:, off, :], start=start, stop=stop)
                elif it[0] == "fir":
                    _, r, stop_fir = it
                    b = r - 1
                    for sdx in (0, 1):
                        for c in (0, 1):
                            src = fsa_sb[:, b + c, :] if sdx == 0 \
                                else fsi_sb[:, b + c, :]
                            nc.tensor.matmul(
                                ps[("r", r)][:], src,
                                fsa_sb[:, 9 + 2 * sdx + c, :],
                                start=False,
                                stop=(stop_fir and sdx == 1 and c == 1))
                elif it[0] == "fin":
                    _, slot = it
                    if slot == ("m", 7):
                        nc.vector.tensor_scalar_mul(st8[:], ps[slot][:],
                                                    1.0 / C_LT)
                    else:
                        nc.vector.tensor_scalar_mul(
                            st[:, SLOT15_COL[slot], :], ps[slot][:],
                            1.0 / C_LT)
                elif it[0] == "store":
                    deferred_stores.append(it)
            # all stores are emitted AFTER the full load stream: the stream
            # is gapless, so a store sitting mid-queue only delays the loads
            # behind it (and with them the whole tail chain)
            for _, lo, hi in deferred_stores:
                eng = nc.sync if n_store % 2 == 0 else nc.scalar
                n_store += 1
                if hi == N_SLOT15:
                    s_inst = eng.dma_start(outm7.ap(), st8[:])
                else:
                    s_inst = eng.dma_start(
                        out2.ap()[:, lo * P:hi * P],
                        st[:, lo:hi, :].rearrange("p j w -> p (j w)"))
                add_dep_helper(s_inst.ins, last_chunk_inst[0].ins, sync=False,
                               reason="stores strictly after the load stream")
            assert unit_i == len(unit_pos)

    nc.compile()
    return nc


def _build_program(dt_name: str):
    if dt_name == "dr15":
        return _build_program15()
    dr8 = dt_name == "dr8"
    lt_dt_name, op_dt_name = _mode_dtypes(dt_name)
    lt_dt = getattr(mybir.dt, lt_dt_name)
    dt = getattr(mybir.dt, op_dt_name)
    npk_dt = mybir.dt.float8e4 if dr8 else dt
    st_dt = mybir.dt.float16 if dr8 else mybir.dt.float32
    f32 = mybir.dt.float32

    nc = bacc.Bacc("TRN2", target_bir_lowering=False, debug=False,
                   num_devices=N_CORES)

    # all inputs are SBUF images: [128 partitions, free...]; lt is a flat
    # sequence of per-chunk SBUF images in consumption order
    lt = nc.dram_tensor("lt", [TOT_KT * P * P], lt_dt, kind="ExternalInput")
    npk = nc.dram_tensor("npk", [P, S // P, P], npk_dt, kind="ExternalInput")
    fs = nc.dram_tensor("fs", [P, N_SLOTS * 2, P], dt, kind="ExternalInput")
    a2 = nc.dram_tensor("a2", [P, 2, 2, P], dt, kind="ExternalInput")
    # single planar output: row = plane*B + batch (plane 0 = real, 1 = imag)
    # -- matches the psum/staging partition layout, so every store is one
    # full-128-partition DMA instead of two 64-partition ones
    out2 = nc.dram_tensor("out2", [2 * B, N_SLOTS * W], st_dt,
                          kind="ExternalOutput")

    with tile.TileContext(nc) as tc:
        with (
            tc.tile_pool(name="const", bufs=1) as const,
            tc.tile_pool(name="ltp", bufs=9) as ltp,
            tc.tile_pool(name="psum", bufs=1, space=bass.MemorySpace.PSUM) as psum,
            tc.tile_pool(name="stage", bufs=1) as stage,
        ):
            # npk streams in window-sized pieces as the pairs consume it; the
            # first pieces go on the scalar ring so chunk 0 leads the sync
            # ring and the first matmul starts as early as possible.
            npk_sb = const.tile([P, S // P, P], npk_dt)
            nc.scalar.dma_start(npk_sb[:, 0:8, :], npk.ap()[:, 0:8, :])
            nc.scalar.dma_start(npk_sb[:, 8:16, :], npk.ap()[:, 8:16, :])
            fs_sb = const.tile([P, N_SLOTS * 2, P], dt)
            a2_sb = const.tile([P, 2, 2, P], dt)
            fsi_sb = const.tile([P, N_SLOTS * 2, P], dt)

            ps = [psum.tile([P, W], f32, name=f"acc{j}", tag=f"acc{j}")
                  for j in range(N_SLOTS)]
            st = stage.tile([P, 6, W], st_dt)
            stB = stage.tile([P, 2, W], st_dt)
            n_dma = 0
            npk_prefetch = {7: (16, 32), 12: (32, 48), 16: (48, 64)}

            def chunk_dma(n_chunk, ck, flat):
                nonlocal n_dma
                ltc = ltp.tile([P, 16, P], lt_dt, tag="lt", name=f"lt{n_chunk}")
                dma_eng = nc.sync if n_dma % 2 == 0 else nc.scalar
                n_dma += 1
                chunk_inst = dma_eng.dma_start(
                    ltc[:, :ck, :],
                    lt.ap()[flat * P * P:(flat + ck) * P * P].rearrange(
                        "(p n m) -> p n m", p=P, n=ck))
                # fs/a2 aren't needed until the first slots complete at the
                # end of pair 0 -- keep them (and the npk prefetches) behind
                # early chunks with explicit edges so the scheduler can't
                # hoist these dep-free const loads ahead of the chunk stream.
                if n_chunk == 3:
                    fs_inst = nc.sync.dma_start(fs_sb[:], fs.ap())
                    add_dep_helper(fs_inst.ins, chunk_inst.ins, sync=False,
                                   reason="defer fs behind first chunk")
                if n_chunk == 4:
                    a2_inst = dma_eng.dma_start(a2_sb[:], a2.ap())
                    add_dep_helper(a2_inst.ins, chunk_inst.ins, sync=False,
                                   reason="defer a2 behind chunk")
                    # slots complete in ascending order -> derive ascending
                    for g in range(N_SLOTS * 2):
                        nc.vector.tensor_scalar_mul(fsi_sb[:, g, 0:B],
                                                    fs_sb[:, g, B:2 * B], -1.0)
                        nc.vector.tensor_copy(fsi_sb[:, g, B:2 * B],
                                              fs_sb[:, g, 0:B])
                # prefetch the next pair's noise window mid-pair
                if n_chunk in npk_prefetch:
                    lo, hi = npk_prefetch[n_chunk]
                    pf_inst = dma_eng.dma_start(npk_sb[:, lo:hi, :],
                                                npk.ap()[:, lo:hi, :])
                    add_dep_helper(pf_inst.ins, chunk_inst.ins, sync=False,
                                   reason="defer npk prefetch behind chunk")
                return ltc

            def fir_mms(j, stop):
                # FIR: stream A_r against [xr|xi], A_i against [-xi|xr]
                for sdx in (0, 1):
                    for c in (0, 1):
                        g = j * 2 + c
                        src = fs_sb if sdx == 0 else fsi_sb
                        nc.tensor.matmul(ps[j][:], src[:, g, :],
                                         a2_sb[:, sdx, c, :],
                                         start=False,
                                         stop=(stop and sdx == 1 and c == 1))

            def finish_slot(j):
                # slot j's accumulation is complete: evacuate and stream out
                # (dr8: the 1/C_LT that undoes the lt pre-scale rides along)
                dst = st[:, j, :] if j < 6 else stB[:, j - 6, :]
                if dr8:
                    nc.vector.tensor_scalar_mul(dst, ps[j][:], 1.0 / C_LT)
                else:
                    nc.vector.tensor_copy(dst, ps[j][:])


            for n_chunk, (j, kt0, ck, flat) in enumerate(CONSUME):
                ltc = chunk_dma(n_chunk, ck, flat)
                # slots 6/7: their FIR only needs fs/a2, so it runs during
                # pair 2, shortening the serial chain after the last chunk
                fir_early = j >= 6 and kt0 == 32
                last_wins_stop = not (j >= 6)
                if dr8:
                    # DoubleRow: one matmul per PAIR of k-tiles (both
                    # operands fp8e4) at 0.5 cycles/row
                    for i in range(0, ck, 2):
                        is_last = kt0 + ck == SLOT_KT[j] and i == ck - 2
                        nc.tensor.matmul(
                            ps[j][:], npk_sb[:, kt0 + i:kt0 + i + 2, :],
                            ltc[:, i:i + 2, :],
                            start=(kt0 + i == 0),
                            stop=(is_last and not last_wins_stop),
                            perf_mode=mybir.MatmulPerfMode.DoubleRow)
                else:
                    for i in range(ck):
                        is_last = kt0 + ck == SLOT_KT[j] and i == ck - 1
                        nc.tensor.matmul(ps[j][:], npk_sb[:, kt0 + i, :],
                                         ltc[:, i, :],
                                         start=(kt0 + i == 0),
                                         stop=(is_last and not last_wins_stop))
                if fir_early:
                    fir_mms(j, stop=False)
                if kt0 + ck == SLOT_KT[j]:
                    if last_wins_stop:
                        fir_mms(j, stop=True)
                    finish_slot(j)
            # all stores emitted after the load stream so they never steal
            # DMA-engine time from the chunk loads; the first two fire as
            # soon as their copies land (in the loads' natural gaps)
            nc.sync.dma_start(out2.ap()[:, :4 * W],
                              st[:, 0:4].rearrange("p j w -> p (j w)"))
            nc.scalar.dma_start(out2.ap()[:, 4 * W:6 * W],
                                st[:, 4:6].rearrange("p j w -> p (j w)"))
            nc.scalar.dma_start(out2.ap()[:, 7 * W:], stB[:, 1, :])
            nc.sync.dma_start(out2.ap()[:, 6 * W:7 * W], stB[:, 0, :])

    nc.compile()
    return nc


def _sbuf_image(arr_ktpm):
    """[nkt*128, m] k-tile-major -> SBUF image [128, nkt*m]."""
    nktp, m = arr_ktpm.shape
    nkt = nktp // P
    return np.ascontiguousarray(
        arr_ktpm.reshape(nkt, P, m).transpose(1, 0, 2).reshape(P, nkt * m))


def _prep_inputs(x_real, x_imag, a_real, a_imag, L, noise_r, noise_i, N0,
                 dt_name: str):
    mixed8 = dt_name == "mixed8"
    dr8 = dt_name == "dr8"
    a2_scale = np.float32(1.0)
    if dr8:
        # lt holds C_LT*L^T in e4m3; noise is NOT pre-divided (it would land
        # in e4m3's subnormal range) -- instead the taps absorb C_LT and the
        # psum evacuation multiplies everything by 1/C_LT.
        import ml_dtypes
        np_dt = np.float16
        npk_np_dt = ml_dtypes.float8_e4m3
        lt_np_dt = ml_dtypes.float8_e4m3
        lt_scale, npk_scale = np.float32(C_LT), np.float32(1.0)
        a2_scale = np.float32(C_LT)
    elif mixed8:
        import ml_dtypes
        np_dt = np.float16
        npk_np_dt = np_dt
        lt_np_dt = ml_dtypes.float8_e3m4
        lt_scale, npk_scale = np.float32(C_LT), np.float32(1.0 / C_LT)
    else:
        np_dt = _DT_NP[dt_name]
        npk_np_dt = np_dt
        lt_np_dt = np_dt
        lt_scale, npk_scale = np.float32(1.0), np.float32(1.0)

    scale = np.float32(np.sqrt(0.5 * np.power(10.0, np.float64(N0[0]) / 10.0)))

    # packed scaled noise [S, 128]: cols 0:64 real, 64:128 imag
    npk = np.empty((S, 2 * B), np.float32)
    npk[:, :B] = (npk_scale * scale * noise_r).T
    npk[:, B:] = (npk_scale * scale * noise_i).T
    npk = _sbuf_image(npk.astype(npk_np_dt)).reshape(P, S // P, P)

    # x transposed and zero-padded by H on both sides: row r <-> x col r - H
    xpad = np.zeros((S + 2 * H, 2 * B), np.float32)
    xpad[H:H + S, :B] = x_real.T
    xpad[H:H + S, B:] = x_imag.T
    xpad = xpad.astype(np_dt)

    # banded Toeplitz of the taps: A[r, j] = a[j + 2H - r] (valid range only)
    a2 = np.zeros((2, 2 * P, P), np.float32)
    rr = np.arange(2 * P)[:, None]
    jj = np.arange(W)[None, :]
    tap_idx = jj + 2 * H - rr
    valid = (tap_idx >= 0) & (tap_idx < T)
    a2[0][valid] = a2_scale * np.asarray(a_real, np.float32)[tap_idx[valid]]
    a2[1][valid] = a2_scale * np.asarray(a_imag, np.float32)[tap_idx[valid]]
    a2 = _sbuf_image(a2.reshape(2 * 2 * P, P).astype(np_dt)).reshape(P, 2, 2, P)

    L = np.asarray(L, np.float32)

    in_maps = []
    for k in range(N_CORES):
        ltpack = np.zeros((TOT_KT * P * P,), lt_np_dt)
        for j, kt0, ck, flat in CONSUME:
            beta = 8 * j + k
            rows_real = P * (beta + 1)     # non-zero extent in t of strip beta
            r0 = P * kt0                   # this chunk covers t rows r0:r1
            nreal = min(max(rows_real - r0, 0), ck * P)
            if nreal <= 0:
                continue
            block = np.zeros((ck * P, W), lt_np_dt)
            block[:nreal] = np.asarray(
                lt_scale * L[P * beta:P * (beta + 1), r0:r0 + nreal],
                lt_np_dt).T
            img = block.reshape(ck, P, W).transpose(1, 0, 2)
            ltpack[flat * P * P:(flat + ck) * P * P] = img.ravel()

        fsk = np.empty((N_SLOTS * 2, P, 2 * B), np_dt)
        for j in range(N_SLOTS):
            s0 = P * (8 * j + k)           # global first output col of slot
            fsk[j * 2] = xpad[s0:s0 + P]           # [xr | xi] k-tile 0
            fsk[j * 2 + 1] = xpad[s0 + P:s0 + 2 * P]  # k-tile 1
        fsk = _sbuf_image(fsk.reshape(N_SLOTS * 2 * P, 2 * B)).reshape(
            P, N_SLOTS * 2, P)
        in_maps.append({"lt": ltpack, "npk": npk, "fs": fsk, "a2": a2})
    return in_maps


def _prep_inputs15(x_real, x_imag, a_real, a_imag, L, noise_r, noise_i, N0):
    import ml_dtypes
    e4 = ml_dtypes.float8_e4m3
    fp16 = np.float16

    scale = np.float32(np.sqrt(0.5 * np.power(10.0, np.float64(N0[0]) / 10.0)))
    Lf = np.asarray(L, np.float32)

    # global scaled noise [S, 2B] in e4m3 (NOT divided by C_LT -- that would
    # land in e4m3's subnormal range; the taps absorb C_LT instead and the
    # psum evacuation multiplies by 1/C_LT)
    npk_g = np.empty((S, 2 * B), np.float32)
    npk_g[:, :B] = (scale * noise_r).T
    npk_g[:, B:] = (scale * noise_i).T
    npk_g = npk_g.astype(e4)

    # x transposed, offset by H=64: xpad[r] = x[r - 64]
    xpad = np.zeros((8320, 2 * B), np.float32)
    xpad[H:H + S, :B] = x_real.T
    xpad[H:H + S, B:] = x_imag.T
    xpad = xpad.astype(fp16)

    # banded Toeplitz taps, pre-scaled by C_LT: 4 images [128, 128]
    a2 = np.zeros((2, 2 * P, P), np.float32)
    rr = np.arange(2 * P)[:, None]
    jj = np.arange(P)[None, :]
    tap_idx = jj + 2 * H - rr
    valid = (tap_idx >= 0) & (tap_idx < T)
    a2[0][valid] = C_LT * np.asarray(a_real, np.float32)[tap_idx[valid]]
    a2[1][valid] = C_LT * np.asarray(a_imag, np.float32)[tap_idx[valid]]
    a2 = a2.reshape(4, P, P).astype(fp16)

    chunk_sizes, _ = _chunks15()
    in_maps = []
    for k in range(N_CORES):
        # npk locals: [0, 8) = globals [8k, 8k+8); [8, 64) = globals [0, 56)
        npk_loc = np.concatenate(
            [npk_g[1024 * k:1024 * (k + 1)], npk_g[:7168]])
        npk_img = _sbuf_image(npk_loc).reshape(P, S // P, P)

        # lt stream: per-unit blocks in consumption order
        tiles = []
        for it in _stream15():
            if it[0] != "mm":
                continue
            _, slot, npk_lo, sz, _, _ = it
            kind, v = slot
            if kind == "m":
                beta = 8 * v + k
                g0 = npk_lo - 8
            else:
                beta = 8 * k + v - 1
                g0 = 8 * k + npk_lo
            blk = (C_LT * Lf[128 * beta:128 * (beta + 1),
                             128 * g0:128 * (g0 + sz)]).T.astype(e4)
            tiles.append(blk.reshape(sz, P, P))
        tiles = np.concatenate(tiles)
        assert tiles.shape[0] == TOT15
        ltpack = np.empty((TOT15 * P * P,), e4)
        flat = 0
        for ck in chunk_sizes:
            ltpack[flat * P * P:(flat + ck) * P * P] = \
                tiles[flat:flat + ck].transpose(1, 0, 2).ravel()
            flat += ck

        # fsa: 9 x-window images + 4 tap images
        fsa = np.empty((13, P, P), fp16)
        for q in range(9):
            fsa[q] = xpad[1024 * k + 128 * q:1024 * k + 128 * (q + 1)]
        fsa[9:13] = a2
        fsa_img = _sbuf_image(fsa.reshape(13 * P, P)).reshape(P, 13, P)

        in_maps.append({"lt": ltpack, "npk": npk_img, "fsa": fsa_img})
    return in_maps


def _unshard15(results):
    """Add top+bottom partials per strip and reassemble [B, S, 2]."""
    outs = []
    for k in range(N_CORES):
        o = np.empty((2 * B, N_SLOT15 * P), np.float32)
        o[:, :(N_SLOT15 - 1) * P] = results[k]["out2"]
        o[:, (N_SLOT15 - 1) * P:] = np.asarray(results[k]["outm7"],
                                               np.float32)
        outs.append(o)
    acc = np.empty((2 * B, S), np.float32)
    for beta in range(64):
        m, r = beta // 8, beta % 8 + 1
        cols = slice(128 * beta, 128 * (beta + 1))
        tc = SLOT15_COL[("r", r)]
        acc[:, cols] = outs[m][:, 128 * tc:128 * (tc + 1)]
        if m >= 1:
            bc = SLOT15_COL[("m", m)]
            acc[:, cols] += outs[beta % 8][:, 128 * bc:128 * (bc + 1)]
    full = np.empty((B, S, 2), np.float32)
    full[:, :, 0] = acc[:B]
    full[:, :, 1] = acc[B:]
    return full


def kernel(x_real, x_imag, a_real, a_imag, L, noise_r, noise_i, N0):
    global LAST_RUN_SECONDS
    inputs = dict(x_real=np.asarray(x_real, np.float32),
                  x_imag=np.asarray(x_imag, np.float32),
                  a_real=np.asarray(a_real, np.float32),
                  a_imag=np.asarray(a_imag, np.float32),
                  L=np.asarray(L, np.float32),
                  noise_r=np.asarray(noise_r, np.float32),
                  noise_i=np.asarray(noise_i, np.float32),
                  N0=np.asarray(N0, np.float32))

    if NOISE_DT not in _CACHE:
        _CACHE[NOISE_DT] = _build_program(NOISE_DT)
    nc = _CACHE[NOISE_DT]

    if NOISE_DT == "dr15":
        in_maps = _prep_inputs15(**inputs)
        t0 = time.time()
        res = run_bass_kernel_spmd(nc, in_maps,
                                   core_ids=list(range(N_CORES)))
        LAST_RUN_SECONDS = time.time() - t0
        return _unshard15(res.results)

    in_maps = _prep_inputs(**inputs, dt_name=NOISE_DT)

    t0 = time.time()
    res = run_bass_kernel_spmd(nc, in_maps, core_ids=list(range(N_CORES)))
    LAST_RUN_SECONDS = time.time() - t0

    planar = np.empty((2, B, N_SLOTS, N_CORES, W), np.float32)
    for k in range(N_CORES):
        o = res.results[k]["out2"].reshape(2, B, N_SLOTS, W)
        planar[0, :, :, k] = o[0]
        planar[1, :, :, k] = o[1]
    full = np.empty((B, S, 2), np.float32)
    full[:, :, 0] = planar[0].reshape(B, S)
    full[:, :, 1] = planar[1].reshape(B, S)
    return full

